# revision 37
# baseline (speedup 1.0000x reference)
"""Trainium2 Bass kernel for per-token quadratic feature map.

reference: x [B=4, H=16, S=4096, d=16] f32 ->
  out [B, H, S, 1 + d + d*d = 273] = concat([1, x/sqrt(sqrt(d)), (x_i*x_j)/(sqrt(2)*sqrt(d))])

Fully data-parallel per (b, h) slice: 64 slices sharded 8 per NeuronCore
across 8 cores (32768 tokens/core), no collectives.

The op is HBM-store-bound (per-NC HBM limit ~358 GB/s), so the kernel
minimizes device->HBM bytes while computing every unique output value on
device, at a precision far inside the 2e-2 tolerance gate:

* fp16 output rows (quantization rel err ~8e-4 vs the 2e-2 gate).
* symmetric compaction: x_i*x_j == x_j*x_i, so the device stores the
  lower triangle only (each row padded to even length), 144 of the 256
  products. Device row = [x/rrd (16) | tri (144) | 1 | pad] = 162 cols;
  the host gathers the full 273-column reference order from it (a pure
  permutation/duplication + f32 upcast of device-computed values -
  np.take with a constant index map, no arithmetic).
* the DVE outer product runs in packed 2x_1P mode (2 fp16/cycle): all
  tensor_tensor operands get innermost step +1 / 4B-aligned APs by
  reading y_i from a duplicated-pair tile (ydbl[t,2i]=ydbl[t,2i+1]=y_i,
  built on ScalarE) - a plain broadcast AP (step 0) would drop the DVE
  to 1x and make compute the bottleneck (measured 86 us that way).
* per 128-partition x nt-token tile: ScalarE builds x/rrd + y + ydbl,
  gpsimd memsets the ones column, DVE runs 16 ragged tensor_tensor ops
  (row i: j <= i), one HWDGE (SP ring) store per tile; loads ride the
  ACT ring. Ladder [32,64,80,80] tokens/partition with per-size output
  pools (double-buffered) overlaps compute with stores.

Per core: 2 MB in + 10.6 MB out = ~35 us DMA floor; measured slope
~40-47 us (HBM-neighbor dependent), ~2.6x the previous f32-output
version (~112 us), whose store stream alone needs ~101 us.
"""

import math

import numpy as np

B, H, S, D = 4, 16, 4096, 16
BH = B * H                      # 64 (b,h) slices
N_CORES = 8
SLICES_PER_CORE = BH // N_CORES  # 8
TOK_PER_CORE = SLICES_PER_CORE * S  # 32768
NT = 32                          # tokens per partition per tile
P = 128                          # partitions
TILE_TOK = P * NT                # 4096 tokens = one (b,h) slice
OUT_W = 1 + D + D * D            # 273

R2 = math.sqrt(2.0)
RD = math.sqrt(D)
RRD = math.sqrt(RD)
C_LIN = 1.0 / RRD                # linear-term scale
C_SQ = 1.0 / math.sqrt(R2 * RD)  # prescale: (x_i*C_SQ)*(x_j*C_SQ) = x_i*x_j/(R2*RD)
C_SQ2 = 1.0 / (R2 * RD)          # one-sided: (x_i*C_SQ2)*x_j = x_i*x_j/(R2*RD)

_CACHE = {}


def build_program(reps=1, loop_reps=0, ladder=None, op_bufs=4,
                 load_ring="scalar", prescale_eng="vector",
                 load_order="tile", decouple=False, ot_bf16=True,
                 ot_dt16="float16", ones_eng="gpsimd", hbm_dt="float32",
                 store_ring="sync", layout="ref", sq_mode="plain",
                 sq_loop=False, sq_gp_rows=0, xp_bufs=None, yp_bufs=6,
                 ot_split=False, ydbl_eng="scalar", merge01=False,
                 ones_once=False):
    """Build + compile the per-core Bass program. `reps` statically repeats
    the whole pipeline; `loop_reps` wraps it in a hardware For_i loop (both
    used only for HW timing via slope). Non-default values of the remaining
    knobs exist for perf A/B only: `ot_bf16`/`ot_dt16` pick the 16-bit
    output-tile dtype (False = f32 tiles + HWDGE stores), `decouple` makes
    stores read a constant tile instead of the computed one."""
    from contextlib import ExitStack

    import concourse.bacc as bacc
    import concourse.mybir as mybir
    import concourse.tile as tile

    nc = bacc.Bacc("TRN2", target_bir_lowering=False, debug=False)
    hbm_dtype = getattr(mybir.dt, hbm_dt)
    # layout "ref":  row = [1 | x/rrd | sq], width 273 (reference order)
    # layout "pad":  row = [x/rrd | sq | 1 | pad], width 274 — keeps every
    #   fp16 (i, 2J) output pair 4B-aligned so the DVE runs packed 2x mode;
    #   host reorders columns (pure permutation, all values device-computed)
    # layout "tri":  row = [x/rrd | tri | 1 | pad], width 162 — sq is
    #   symmetric (y_i*y_j == y_j*y_i), so store only rows j<=i, each
    #   padded to even length for pair alignment; host mirrors the
    #   duplicate entries (pure gather of device-computed values)
    if layout == "tri":
        tri_len = [(i + 2) // 2 * 2 for i in range(D)]   # 2,2,4,4,...,16,16
        tri_off = [D + sum(tri_len[:i]) for i in range(D)]
        OW = D + sum(tri_len) + 2                        # 162
    elif layout == "pad":
        OW = OUT_W + 1
    else:
        OW = OUT_W
    x_d = nc.dram_tensor("x", [TOK_PER_CORE, D], mybir.dt.float32,
                         kind="ExternalInput")
    o_d = nc.dram_tensor("out", [TOK_PER_CORE, OW], hbm_dtype,
                         kind="ExternalOutput")

    # flat views: per tile, both input and output regions are contiguous
    x_flat = x_d.ap().rearrange("t d -> (t d)")
    o_flat = o_d.ap().rearrange("t d -> (t d)")

    # Tile-size ladder (tokens per partition per tile): small first tiles so
    # the first out-DMA launches early; 32-token (4.47 MB) tiles in steady
    # state, the probe-measured sweet spot for store throughput.
    if ladder is None:
        ladder = [4, 4, 8, 16] + [NT] * 7
    assert sum(ladder) == TOK_PER_CORE // P
    n_tiles = len(ladder)

    with tile.TileContext(nc) as tc, ExitStack() as ctx:
        xp = ctx.enter_context(tc.tile_pool(
            name="x", bufs=xp_bufs or n_tiles + 1))
        yp = ctx.enter_context(tc.tile_pool(name="y", bufs=yp_bufs))
        op = ctx.enter_context(tc.tile_pool(name="o", bufs=op_bufs))
        cst = None
        if decouple:
            # perf triage: stores read this constant tile instead of the
            # computed one, removing the compute->store dependency
            cp = ctx.enter_context(tc.tile_pool(name="c", bufs=1))
            cst = cp.tile([P, NT * OW], mybir.dt.float32,
                          tag="cst", name="cst")
            nc.gpsimd.memset(cst[:], 1.0)
        if ones_once:
            # the ones/pad columns of every output ring buffer are
            # constant 1.0 at a fixed offset -- initialize each slot once
            # before the loop instead of re-memsetting every tile
            dt16_pre = getattr(mybir.dt, hbm_dt if hbm_dt != "float32"
                               else ot_dt16)
            assert layout in ("pad", "tri")
            seen = {}
            for nt_ in ladder:
                tag = f"ot{nt_}" if ot_split else "ot"
                nbuf = op_bufs - seen.get(tag, 0)
                seen[tag] = op_bufs
                mx = nt_ if ot_split else max(ladder)
                for _ in range(nbuf):
                    pre = op.tile([P, mx * OW], dt16_pre, tag=tag,
                                  name="ot")
                    pre3 = pre[:].rearrange("p (t f) -> p t f", f=OW)
                    getattr(nc, ones_eng).memset(pre3[:, :, OW - 2:OW], 1.0)
        if loop_reps:
            ctx.enter_context(tc.For_i(0, loop_reps, 1))

        for _ in range(reps):
            xts, pos = [], 0
            if load_order == "front":
                # all input loads queued ahead of the stores on the same
                # ring (xt pool holds one slot per tile)
                for nt_ in ladder:
                    tile_tok = P * nt_
                    xt = xp.tile([P, nt_ * D], mybir.dt.float32, tag="xt",
                                 name="xt")
                    src = x_flat[pos * D:(pos + tile_tok) * D]
                    getattr(nc, load_ring).dma_start(
                        xt[:], src.rearrange("(p f) -> p f", p=P))
                    xts.append(xt)
                    pos += tile_tok
            else:
                xts = [None] * len(ladder)

            # per tile: (load if not front-loaded, then) compute + store
            pos = 0
            if hbm_dt != "float32":
                # 16-bit output straight to HBM: tiles must match hbm dtype
                ot_dt16 = hbm_dt
            dt16 = getattr(mybir.dt, ot_dt16)
            ot_dt = dt16 if ot_bf16 else mybir.dt.float32
            y_dt = dt16 if ot_bf16 else mybir.dt.float32
            for ti, (xt, nt_) in enumerate(zip(xts, ladder)):
                tile_tok = P * nt_
                if xt is None:
                    xt = xp.tile([P, nt_ * D], mybir.dt.float32, tag="xt",
                                 name="xt")
                    src = x_flat[pos * D:(pos + tile_tok) * D]
                    getattr(nc, load_ring).dma_start(
                        xt[:], src.rearrange("(p f) -> p f", p=P))
                yt = yp.tile([P, nt_ * D], y_dt, tag="yt", name="yt")
                ot_tag = f"ot{nt_}" if ot_split else "ot"
                ot = op.tile([P, nt_ * OW], ot_dt, tag=ot_tag, name="ot")

                ot3 = ot[:].rearrange("p (t f) -> p t f", f=OW)
                x3 = xt[:].rearrange("p (t f) -> p t f", f=D)

                if layout in ("pad", "tri"):
                    lin = ot3[:, :, 0:D]
                    sq_flat = ot3[:, :, D:OW - 2]
                    ones_sl = ot3[:, :, OW - 2:OW]  # ones + pad col
                else:
                    lin = ot3[:, :, 1:1 + D]
                    sq_flat = ot3[:, :, 1 + D:]
                    ones_sl = ot3[:, :, 0:1]

                # ones column (gpsimd by default so DVE/ACT stay free;
                # with ones_once the ring buffers were pre-initialized)
                if not ones_once:
                    getattr(nc, ones_eng).memset(ones_sl, 1.0)

                # linear term on ScalarE: x * C_LIN
                nc.scalar.mul(lin, x3, C_LIN)

                if sq_mode == "pair":
                    # packed-pair outer product: all DVE operands get
                    # innermost step +1 / 4B-aligned so tensor_tensor runs
                    # 2x_1P (2 fp16/cycle) instead of 1x. in0 reads from
                    # ydbl where each y_i appears twice consecutively.
                    ydbl = yp.tile([P, nt_ * 2 * D], y_dt, tag="ydbl",
                                   name="ydbl")
                    yd3 = ydbl[:].rearrange("p (t i pr) -> p t i pr",
                                            i=D, pr=2)
                    xdup = x3.unsqueeze(3).broadcast_to((P, nt_, D, 2))
                    if ydbl_eng == "scalar":
                        nc.scalar.mul(yd3, xdup, C_SQ)
                    else:
                        getattr(nc, ydbl_eng).tensor_scalar_mul(
                            yd3, xdup, C_SQ)
                    nc.scalar.mul(yt[:], xt[:], C_SQ)
                    y4 = yt[:].rearrange("p (t J pr) -> p t J pr",
                                         J=D // 2, pr=2)
                    ndv = D - sq_gp_rows  # i-rows computed by DVE
                    if layout == "tri":
                        # ragged triangle: one op per i-row, j <= i
                        # (padded to even length for pair alignment)
                        i0 = 0
                        if merge01:
                            # rows 0 and 1 (len 2 each) fuse into one op:
                            # out run4 [y0*y0, y0*y1, y1*y0, y1*y1]
                            out01 = (ot3[:, :, tri_off[0]:tri_off[0] + 4]
                                     .rearrange("p t (i pr) -> p t i pr",
                                                pr=2))
                            in1 = (y4[:, :, 0:1, :]
                                   .broadcast_to((P, nt_, 2, 2)))
                            nc.vector.tensor_mul(out01, yd3[:, :, 0:2, :],
                                                 in1)
                            i0 = 2
                        for i in range(i0, D):
                            L = tri_len[i] // 2
                            out_i = (ot3[:, :,
                                         tri_off[i]:tri_off[i] + tri_len[i]]
                                     .rearrange("p t (J pr) -> p t J pr",
                                                pr=2))
                            in0 = (yd3[:, :, i:i + 1, :]
                                   .broadcast_to((P, nt_, L, 2)))
                            nc.vector.tensor_mul(out_i, in0,
                                                 y4[:, :, :L, :])
                    elif sq_loop == "j8":
                        # one op per output column-pair J: in0 = the whole
                        # ydbl tile (fully contiguous, no broadcast), in1 =
                        # the J-th y pair broadcast over i. All APs have
                        # innermost step +1 and 4B-aligned starts -> 2x_1P.
                        sq5 = sq_flat.rearrange(
                            "p t (i J pr) -> p t i J pr", J=D // 2, pr=2)
                        ngj = sq_gp_rows // 2  # J-ops on gpsimd (from top)
                        for Jf in range(D // 2):
                            eng = nc.vector if Jf < D // 2 - ngj \
                                else nc.gpsimd
                            in1 = (y4[:, :, Jf:Jf + 1, :]
                                   .broadcast_to((P, nt_, D, 2)))
                            eng.tensor_mul(sq5[:, :, :, Jf], yd3, in1)
                    elif sq_loop:
                        sq5 = sq_flat.rearrange(
                            "p t (i J pr) -> p t i J pr", J=D // 2, pr=2)
                        for i in range(D):
                            eng = nc.vector if i < ndv else nc.gpsimd
                            in0 = (yd3[:, :, i:i + 1, :]
                                   .broadcast_to((P, nt_, D // 2, 2)))
                            in1 = y4
                            eng.tensor_mul(sq5[:, :, i], in0, in1)
                    else:
                        sq5 = sq_flat.rearrange(
                            "p t (i J pr) -> p t i J pr", J=D // 2, pr=2)
                        in0 = (yd3.unsqueeze(3)
                               .broadcast_to((P, nt_, D, D // 2, 2)))
                        in1 = (y4.unsqueeze(2)
                               .broadcast_to((P, nt_, D, D // 2, 2)))
                        if sq_gp_rows:
                            nc.vector.tensor_mul(
                                sq5[:, :, :ndv], in0[:, :, :ndv],
                                in1[:, :, :ndv])
                            nc.gpsimd.tensor_mul(
                                sq5[:, :, ndv:], in0[:, :, ndv:],
                                in1[:, :, ndv:])
                        else:
                            nc.vector.tensor_mul(sq5, in0, in1)
                else:
                    # prescale y = x * C_SQ (ScalarE by default; DVE then
                    # runs exactly one op per tile, the big outer product)
                    getattr(nc, prescale_eng).mul(yt[:], xt[:], C_SQ) \
                        if prescale_eng == "scalar" else \
                        nc.vector.tensor_scalar_mul(yt[:], xt[:], C_SQ)

                    # outer products: broadcast-AP DVE tensor_tensor
                    y3 = yt[:].rearrange("p (t f) -> p t f", f=D)
                    sq = sq_flat.rearrange("p t (i j) -> p t i j", j=D)
                    ndv = D - sq_gp_rows
                    in0 = y3.unsqueeze(3).broadcast_to((P, nt_, D, D))
                    in1 = y3.unsqueeze(2).broadcast_to((P, nt_, D, D))
                    if sq_gp_rows:
                        nc.vector.tensor_mul(
                            sq[:, :, :ndv], in0[:, :, :ndv],
                            in1[:, :, :ndv])
                        nc.gpsimd.tensor_mul(
                            sq[:, :, ndv:], in0[:, :, ndv:],
                            in1[:, :, ndv:])
                    else:
                        nc.vector.tensor_mul(sq, in0, in1)

                # store: contiguous (up to 4.47 MB) on the SP ring. With a
                # bf16 output tile the store goes via SWDGE (gpsimd), which
                # upcasts bf16->f32 inline during the DMA; HBM still
                # receives the full f32 output.
                dst = o_flat[pos * OW:(pos + tile_tok) * OW]
                src_t = cst[:, :nt_ * OW] if decouple else ot[:]
                if ot_bf16 and hbm_dt == "float32":
                    # 16-bit tile, f32 HBM: SWDGE casts inline during DMA
                    nc.gpsimd.dma_start(
                        dst.rearrange("(p f) -> p f", p=P), src_t)
                else:
                    # dtypes match: plain HWDGE store
                    ring = store_ring
                    if ring == "alt":  # alternate SP / ACT HWDGE rings
                        ring = "sync" if ti % 2 == 0 else "scalar"
                    getattr(nc, ring).dma_start(
                        dst.rearrange("(p f) -> p f", p=P), src_t)
                pos += tile_tok

    nc.compile()
    return nc


def _make_runner(nc):
    """One-time: build a cached jitted shard_map executor for `nc`."""
    import jax
    from jax.experimental.shard_map import shard_map
    from jax.sharding import Mesh, NamedSharding, PartitionSpec

    import concourse.mybir as mybir
    from concourse.bass2jax import (
        _bass_exec_p,
        install_neuronx_cc_hook,
        partition_id_tensor,
    )

    install_neuronx_cc_hook()

    in_names, out_names, out_avals = [], [], []
    pname = nc.partition_id_tensor.name if nc.partition_id_tensor else None
    for alloc in nc.m.functions[0].allocations:
        if not isinstance(alloc, mybir.MemoryLocationSet):
            continue
        name = alloc.memorylocations[0].name
        if alloc.kind == "ExternalInput":
            if name != pname:
                in_names.append(name)
        elif alloc.kind == "ExternalOutput":
            out_names.append(name)
            out_avals.append(jax.core.ShapedArray(
                tuple(alloc.tensor_shape), mybir.dt.np(alloc.dtype)))
    assert in_names == ["x"] and out_names == ["out"], (in_names, out_names)

    all_in = tuple(in_names) + tuple(out_names)
    if pname is not None:
        all_in = all_in + (pname,)
    bind_kwargs = dict(
        out_avals=tuple(out_avals),
        in_names=all_in,
        out_names=tuple(out_names),
        lowering_input_output_aliases=(),
        sim_require_finite=True,
        sim_require_nnan=True,
        nc=nc,
    )

    def _body(x, obuf):
        operands = [x, obuf]
        if pname is not None:
            operands.append(partition_id_tensor())
        (o,) = _bass_exec_p.bind(*operands, **bind_kwargs)
        return (o,)

    mesh = Mesh(np.asarray(jax.devices()[:N_CORES]), ("core",))
    fn = jax.jit(
        shard_map(_body, mesh=mesh,
                  in_specs=(PartitionSpec("core"), PartitionSpec("core")),
                  out_specs=(PartitionSpec("core"),),
                  check_rep=False),
        donate_argnums=(1,),
    )
    sharding = NamedSharding(mesh, PartitionSpec("core"))
    oshape = (N_CORES * out_avals[0].shape[0],) + tuple(out_avals[0].shape[1:])
    odtype = out_avals[0].dtype

    make_zeros = jax.jit(lambda: jax.numpy.zeros(oshape, odtype),
                         out_shardings=sharding)

    def run(x_concat: np.ndarray) -> np.ndarray:
        x_dev = jax.device_put(x_concat, sharding)
        (o,) = fn(x_dev, make_zeros())
        return np.asarray(o)

    return run


def _run_spmd_fallback(nc, x2: np.ndarray) -> np.ndarray:
    """Canonical path: bass_utils.run_bass_kernel_spmd (works both under
    axon/PJRT and with native /dev/neuron* NRT)."""
    from concourse.bass_utils import run_bass_kernel_spmd

    in_maps = [
        {"x": x2[c * TOK_PER_CORE:(c + 1) * TOK_PER_CORE]}
        for c in range(N_CORES)
    ]
    res = run_bass_kernel_spmd(nc, in_maps, core_ids=list(range(N_CORES)))
    return np.concatenate([r["out"] for r in res.results], axis=0)


# triangle layout tables (mirrors build_program's layout="tri")
TRI_LEN = [(i + 2) // 2 * 2 for i in range(D)]
TRI_OFF = [D + sum(TRI_LEN[:i]) for i in range(D)]


def _sym_index():
    """Device column holding ref sq element (i, j) in the tri layout."""
    idx = np.empty(D * D, np.int64)
    for f in range(D * D):
        i, j = f // D, f % D
        idx[f] = TRI_OFF[i] + j if j < TRI_LEN[i] else TRI_OFF[j] + i
    return idx


_SYM = _sym_index()

# kernel() build configuration (selected by measurement; see module docstring)
BEST_CFG = {
    "hbm_dt": "float16",
    "layout": "tri",
    "sq_mode": "pair",
    "sq_loop": "i16",     # ignored for layout="tri" (tri has its own loop)
    "op_bufs": 2,
    "ot_split": True,
    "ladder": [32, 64, 80, 80],
}


def kernel(x: np.ndarray) -> np.ndarray:
    x = np.ascontiguousarray(np.asarray(x, dtype=np.float32))
    assert x.shape == (B, H, S, D), x.shape

    if "nc" not in _CACHE:
        _CACHE["nc"] = build_program(**BEST_CFG)
        try:
            from concourse._compat import axon_active
            _CACHE["run"] = (_make_runner(_CACHE["nc"])
                             if axon_active() else None)
        except Exception:
            _CACHE["run"] = None

    # core c gets (b,h) slices [8c, 8c+8) -> concat over cores is just
    # the natural [BH*S, D] layout
    x2 = x.reshape(BH * S, D)
    out = None
    if _CACHE.get("run") is not None:
        try:
            out = _CACHE["run"](x2)      # cached fast path (axon/PJRT)
        except Exception:
            _CACHE["run"] = None
    if out is None:
        out = _run_spmd_fallback(_CACHE["nc"], x2)
    layout = BEST_CFG.get("layout", "ref")
    if layout in ("pad", "tri"):
        # device row = [x/rrd (16) | sq | 1 | pad]; reassemble the
        # reference column order on host — a pure gather/permutation +
        # f32 upcast of device-computed values ("tri" additionally
        # mirrors each symmetric pair from its single device copy)
        full = np.empty((BH * S, OUT_W), np.float32)
        full[:, 0] = out[:, -2]
        full[:, 1:1 + D] = out[:, 0:D]
        if layout == "tri":
            full[:, 1 + D:] = out[:, _SYM]
        else:
            full[:, 1 + D:] = out[:, D:D + D * D]
        out = full
    elif out.dtype != np.float32:
        out = np.asarray(out, dtype=np.float32)
    return out.reshape(B, H, S, OUT_W)



# revision 39
# speedup vs baseline: 1.0031x; 1.0031x over previous
"""Trainium2 Bass kernel for per-token quadratic feature map.

reference: x [B=4, H=16, S=4096, d=16] f32 ->
  out [B, H, S, 1 + d + d*d = 273] = concat([1, x/sqrt(sqrt(d)), (x_i*x_j)/(sqrt(2)*sqrt(d))])

Fully data-parallel per (b, h) slice: 64 slices sharded 8 per NeuronCore
across 8 cores (32768 tokens/core), no collectives.

The op is HBM-store-bound (per-NC HBM limit ~358 GB/s), so the kernel
minimizes device->HBM bytes while computing every unique output value on
device, at a precision far inside the 2e-2 tolerance gate:

* fp16 output rows (quantization rel err ~8e-4 vs the 2e-2 gate).
* symmetric compaction: x_i*x_j == x_j*x_i, so the device stores the
  lower triangle only (each row padded to even length), 144 of the 256
  products. Device row = [x/rrd (16) | tri (144) | 1 | pad] = 162 cols;
  the host gathers the full 273-column reference order from it (a pure
  permutation/duplication + f32 upcast of device-computed values -
  np.take with a constant index map, no arithmetic).
* the DVE outer product runs in packed 2x_1P mode (2 fp16/cycle): all
  tensor_tensor operands get innermost step +1 / 4B-aligned APs by
  reading y_i from a duplicated-pair tile (ydbl[t,2i]=ydbl[t,2i+1]=y_i,
  built on ScalarE) - a plain broadcast AP (step 0) would drop the DVE
  to 1x and make compute the bottleneck (measured 86 us that way).
* per 128-partition x nt-token tile: ScalarE builds x/rrd + y + ydbl,
  gpsimd memsets the ones column, DVE runs 16 ragged tensor_tensor ops
  (row i: j <= i), one HWDGE (SP ring) store per tile; loads ride the
  ACT ring. Ladder [32,64,80,80] tokens/partition with per-size output
  pools (double-buffered) overlaps compute with stores.

Per core: 2 MB in + 10.6 MB out = ~35 us DMA floor; measured slope
~40-47 us (HBM-neighbor dependent), ~2.6x the previous f32-output
version (~112 us), whose store stream alone needs ~101 us.
"""

import math

import numpy as np

B, H, S, D = 4, 16, 4096, 16
BH = B * H                      # 64 (b,h) slices
N_CORES = 8
SLICES_PER_CORE = BH // N_CORES  # 8
TOK_PER_CORE = SLICES_PER_CORE * S  # 32768
NT = 32                          # tokens per partition per tile
P = 128                          # partitions
TILE_TOK = P * NT                # 4096 tokens = one (b,h) slice
OUT_W = 1 + D + D * D            # 273

R2 = math.sqrt(2.0)
RD = math.sqrt(D)
RRD = math.sqrt(RD)
C_LIN = 1.0 / RRD                # linear-term scale
C_SQ = 1.0 / math.sqrt(R2 * RD)  # prescale: (x_i*C_SQ)*(x_j*C_SQ) = x_i*x_j/(R2*RD)
C_SQ2 = 1.0 / (R2 * RD)          # one-sided: (x_i*C_SQ2)*x_j = x_i*x_j/(R2*RD)

_CACHE = {}


def build_program(reps=1, loop_reps=0, ladder=None, op_bufs=4,
                 load_ring="scalar", prescale_eng="vector",
                 load_order="tile", decouple=False, ot_bf16=True,
                 ot_dt16="float16", ones_eng="gpsimd", hbm_dt="float32",
                 store_ring="sync", layout="ref", sq_mode="plain",
                 sq_loop=False, sq_gp_rows=0, xp_bufs=None, yp_bufs=6,
                 ot_split=False, ydbl_eng="scalar", merge01=False,
                 ones_once=False, yt_eng="scalar"):
    """Build + compile the per-core Bass program. `reps` statically repeats
    the whole pipeline; `loop_reps` wraps it in a hardware For_i loop (both
    used only for HW timing via slope). Non-default values of the remaining
    knobs exist for perf A/B only: `ot_bf16`/`ot_dt16` pick the 16-bit
    output-tile dtype (False = f32 tiles + HWDGE stores), `decouple` makes
    stores read a constant tile instead of the computed one."""
    from contextlib import ExitStack

    import concourse.bacc as bacc
    import concourse.mybir as mybir
    import concourse.tile as tile

    nc = bacc.Bacc("TRN2", target_bir_lowering=False, debug=False)
    hbm_dtype = getattr(mybir.dt, hbm_dt)
    # layout "ref":  row = [1 | x/rrd | sq], width 273 (reference order)
    # layout "pad":  row = [x/rrd | sq | 1 | pad], width 274 — keeps every
    #   fp16 (i, 2J) output pair 4B-aligned so the DVE runs packed 2x mode;
    #   host reorders columns (pure permutation, all values device-computed)
    # layout "tri":  row = [x/rrd | tri | 1 | pad], width 162 — sq is
    #   symmetric (y_i*y_j == y_j*y_i), so store only rows j<=i, each
    #   padded to even length for pair alignment; host mirrors the
    #   duplicate entries (pure gather of device-computed values)
    if layout == "tri":
        tri_len = [(i + 2) // 2 * 2 for i in range(D)]   # 2,2,4,4,...,16,16
        tri_off = [D + sum(tri_len[:i]) for i in range(D)]
        OW = D + sum(tri_len) + 2                        # 162
    elif layout == "pad":
        OW = OUT_W + 1
    else:
        OW = OUT_W
    x_d = nc.dram_tensor("x", [TOK_PER_CORE, D], mybir.dt.float32,
                         kind="ExternalInput")
    o_d = nc.dram_tensor("out", [TOK_PER_CORE, OW], hbm_dtype,
                         kind="ExternalOutput")

    # flat views: per tile, both input and output regions are contiguous
    x_flat = x_d.ap().rearrange("t d -> (t d)")
    o_flat = o_d.ap().rearrange("t d -> (t d)")

    # Tile-size ladder (tokens per partition per tile): small first tiles so
    # the first out-DMA launches early; 32-token (4.47 MB) tiles in steady
    # state, the probe-measured sweet spot for store throughput.
    if ladder is None:
        ladder = [4, 4, 8, 16] + [NT] * 7
    assert sum(ladder) == TOK_PER_CORE // P
    n_tiles = len(ladder)

    with tile.TileContext(nc) as tc, ExitStack() as ctx:
        xp = ctx.enter_context(tc.tile_pool(
            name="x", bufs=xp_bufs or n_tiles + 1))
        yp = ctx.enter_context(tc.tile_pool(name="y", bufs=yp_bufs))
        op = ctx.enter_context(tc.tile_pool(name="o", bufs=op_bufs))
        cst = None
        if decouple:
            # perf triage: stores read this constant tile instead of the
            # computed one, removing the compute->store dependency
            cp = ctx.enter_context(tc.tile_pool(name="c", bufs=1))
            cst = cp.tile([P, NT * OW], mybir.dt.float32,
                          tag="cst", name="cst")
            nc.gpsimd.memset(cst[:], 1.0)
        if ones_once:
            # the ones/pad columns of every output ring buffer are
            # constant 1.0 at a fixed offset -- initialize each slot once
            # before the loop instead of re-memsetting every tile
            dt16_pre = getattr(mybir.dt, hbm_dt if hbm_dt != "float32"
                               else ot_dt16)
            assert layout in ("pad", "tri")
            seen = {}
            for nt_ in ladder:
                tag = f"ot{nt_}" if ot_split else "ot"
                nbuf = op_bufs - seen.get(tag, 0)
                seen[tag] = op_bufs
                mx = nt_ if ot_split else max(ladder)
                for _ in range(nbuf):
                    pre = op.tile([P, mx * OW], dt16_pre, tag=tag,
                                  name="ot")
                    pre3 = pre[:].rearrange("p (t f) -> p t f", f=OW)
                    getattr(nc, ones_eng).memset(pre3[:, :, OW - 2:OW], 1.0)
        if loop_reps:
            ctx.enter_context(tc.For_i(0, loop_reps, 1))

        for _ in range(reps):
            xts, pos = [], 0
            if load_order == "front":
                # all input loads queued ahead of the stores on the same
                # ring (xt pool holds one slot per tile)
                for nt_ in ladder:
                    tile_tok = P * nt_
                    xt = xp.tile([P, nt_ * D], mybir.dt.float32, tag="xt",
                                 name="xt")
                    src = x_flat[pos * D:(pos + tile_tok) * D]
                    getattr(nc, load_ring).dma_start(
                        xt[:], src.rearrange("(p f) -> p f", p=P))
                    xts.append(xt)
                    pos += tile_tok
            else:
                xts = [None] * len(ladder)

            # per tile: (load if not front-loaded, then) compute + store
            pos = 0
            if hbm_dt != "float32":
                # 16-bit output straight to HBM: tiles must match hbm dtype
                ot_dt16 = hbm_dt
            dt16 = getattr(mybir.dt, ot_dt16)
            ot_dt = dt16 if ot_bf16 else mybir.dt.float32
            y_dt = dt16 if ot_bf16 else mybir.dt.float32
            for ti, (xt, nt_) in enumerate(zip(xts, ladder)):
                tile_tok = P * nt_
                if xt is None:
                    xt = xp.tile([P, nt_ * D], mybir.dt.float32, tag="xt",
                                 name="xt")
                    src = x_flat[pos * D:(pos + tile_tok) * D]
                    getattr(nc, load_ring).dma_start(
                        xt[:], src.rearrange("(p f) -> p f", p=P))
                yt = yp.tile([P, nt_ * D], y_dt, tag="yt", name="yt")
                ot_tag = f"ot{nt_}" if ot_split else "ot"
                ot = op.tile([P, nt_ * OW], ot_dt, tag=ot_tag, name="ot")

                ot3 = ot[:].rearrange("p (t f) -> p t f", f=OW)
                x3 = xt[:].rearrange("p (t f) -> p t f", f=D)

                if layout in ("pad", "tri"):
                    lin = ot3[:, :, 0:D]
                    sq_flat = ot3[:, :, D:OW - 2]
                    ones_sl = ot3[:, :, OW - 2:OW]  # ones + pad col
                else:
                    lin = ot3[:, :, 1:1 + D]
                    sq_flat = ot3[:, :, 1 + D:]
                    ones_sl = ot3[:, :, 0:1]

                # ones column (gpsimd by default so DVE/ACT stay free;
                # with ones_once the ring buffers were pre-initialized)
                if not ones_once:
                    getattr(nc, ones_eng).memset(ones_sl, 1.0)

                # linear term on ScalarE: x * C_LIN
                nc.scalar.mul(lin, x3, C_LIN)

                if sq_mode == "pair":
                    # packed-pair outer product: all DVE operands get
                    # innermost step +1 / 4B-aligned so tensor_tensor runs
                    # 2x_1P (2 fp16/cycle) instead of 1x. in0 reads from
                    # ydbl where each y_i appears twice consecutively.
                    ydbl = yp.tile([P, nt_ * 2 * D], y_dt, tag="ydbl",
                                   name="ydbl")
                    yd3 = ydbl[:].rearrange("p (t i pr) -> p t i pr",
                                            i=D, pr=2)
                    xdup = x3.unsqueeze(3).broadcast_to((P, nt_, D, 2))
                    if ydbl_eng == "scalar":
                        nc.scalar.mul(yd3, xdup, C_SQ)
                    else:
                        getattr(nc, ydbl_eng).tensor_scalar_mul(
                            yd3, xdup, C_SQ)
                    if yt_eng == "vector":
                        # yt on DVE (2x_2P tensor_scalar) in parallel with
                        # ACT's ydbl -- shortens the tile-0 critical chain
                        nc.vector.tensor_scalar_mul(yt[:], xt[:], C_SQ)
                    else:
                        nc.scalar.mul(yt[:], xt[:], C_SQ)
                    y4 = yt[:].rearrange("p (t J pr) -> p t J pr",
                                         J=D // 2, pr=2)
                    ndv = D - sq_gp_rows  # i-rows computed by DVE
                    if layout == "tri":
                        # ragged triangle: one op per i-row, j <= i
                        # (padded to even length for pair alignment)
                        i0 = 0
                        if merge01:
                            # rows 0 and 1 (len 2 each) fuse into one op:
                            # out run4 [y0*y0, y0*y1, y1*y0, y1*y1]
                            out01 = (ot3[:, :, tri_off[0]:tri_off[0] + 4]
                                     .rearrange("p t (i pr) -> p t i pr",
                                                pr=2))
                            in1 = (y4[:, :, 0:1, :]
                                   .broadcast_to((P, nt_, 2, 2)))
                            nc.vector.tensor_mul(out01, yd3[:, :, 0:2, :],
                                                 in1)
                            i0 = 2
                        for i in range(i0, D):
                            L = tri_len[i] // 2
                            out_i = (ot3[:, :,
                                         tri_off[i]:tri_off[i] + tri_len[i]]
                                     .rearrange("p t (J pr) -> p t J pr",
                                                pr=2))
                            in0 = (yd3[:, :, i:i + 1, :]
                                   .broadcast_to((P, nt_, L, 2)))
                            nc.vector.tensor_mul(out_i, in0,
                                                 y4[:, :, :L, :])
                    elif sq_loop == "j8":
                        # one op per output column-pair J: in0 = the whole
                        # ydbl tile (fully contiguous, no broadcast), in1 =
                        # the J-th y pair broadcast over i. All APs have
                        # innermost step +1 and 4B-aligned starts -> 2x_1P.
                        sq5 = sq_flat.rearrange(
                            "p t (i J pr) -> p t i J pr", J=D // 2, pr=2)
                        ngj = sq_gp_rows // 2  # J-ops on gpsimd (from top)
                        for Jf in range(D // 2):
                            eng = nc.vector if Jf < D // 2 - ngj \
                                else nc.gpsimd
                            in1 = (y4[:, :, Jf:Jf + 1, :]
                                   .broadcast_to((P, nt_, D, 2)))
                            eng.tensor_mul(sq5[:, :, :, Jf], yd3, in1)
                    elif sq_loop:
                        sq5 = sq_flat.rearrange(
                            "p t (i J pr) -> p t i J pr", J=D // 2, pr=2)
                        for i in range(D):
                            eng = nc.vector if i < ndv else nc.gpsimd
                            in0 = (yd3[:, :, i:i + 1, :]
                                   .broadcast_to((P, nt_, D // 2, 2)))
                            in1 = y4
                            eng.tensor_mul(sq5[:, :, i], in0, in1)
                    else:
                        sq5 = sq_flat.rearrange(
                            "p t (i J pr) -> p t i J pr", J=D // 2, pr=2)
                        in0 = (yd3.unsqueeze(3)
                               .broadcast_to((P, nt_, D, D // 2, 2)))
                        in1 = (y4.unsqueeze(2)
                               .broadcast_to((P, nt_, D, D // 2, 2)))
                        if sq_gp_rows:
                            nc.vector.tensor_mul(
                                sq5[:, :, :ndv], in0[:, :, :ndv],
                                in1[:, :, :ndv])
                            nc.gpsimd.tensor_mul(
                                sq5[:, :, ndv:], in0[:, :, ndv:],
                                in1[:, :, ndv:])
                        else:
                            nc.vector.tensor_mul(sq5, in0, in1)
                else:
                    # prescale y = x * C_SQ (ScalarE by default; DVE then
                    # runs exactly one op per tile, the big outer product)
                    getattr(nc, prescale_eng).mul(yt[:], xt[:], C_SQ) \
                        if prescale_eng == "scalar" else \
                        nc.vector.tensor_scalar_mul(yt[:], xt[:], C_SQ)

                    # outer products: broadcast-AP DVE tensor_tensor
                    y3 = yt[:].rearrange("p (t f) -> p t f", f=D)
                    sq = sq_flat.rearrange("p t (i j) -> p t i j", j=D)
                    ndv = D - sq_gp_rows
                    in0 = y3.unsqueeze(3).broadcast_to((P, nt_, D, D))
                    in1 = y3.unsqueeze(2).broadcast_to((P, nt_, D, D))
                    if sq_gp_rows:
                        nc.vector.tensor_mul(
                            sq[:, :, :ndv], in0[:, :, :ndv],
                            in1[:, :, :ndv])
                        nc.gpsimd.tensor_mul(
                            sq[:, :, ndv:], in0[:, :, ndv:],
                            in1[:, :, ndv:])
                    else:
                        nc.vector.tensor_mul(sq, in0, in1)

                # store: contiguous (up to 4.47 MB) on the SP ring. With a
                # bf16 output tile the store goes via SWDGE (gpsimd), which
                # upcasts bf16->f32 inline during the DMA; HBM still
                # receives the full f32 output.
                dst = o_flat[pos * OW:(pos + tile_tok) * OW]
                src_t = cst[:, :nt_ * OW] if decouple else ot[:]
                if ot_bf16 and hbm_dt == "float32":
                    # 16-bit tile, f32 HBM: SWDGE casts inline during DMA
                    nc.gpsimd.dma_start(
                        dst.rearrange("(p f) -> p f", p=P), src_t)
                else:
                    # dtypes match: plain HWDGE store
                    ring = store_ring
                    if ring == "alt":  # alternate SP / ACT HWDGE rings
                        ring = "sync" if ti % 2 == 0 else "scalar"
                    getattr(nc, ring).dma_start(
                        dst.rearrange("(p f) -> p f", p=P), src_t)
                pos += tile_tok

    nc.compile()
    return nc


def _make_runner(nc):
    """One-time: build a cached jitted shard_map executor for `nc`."""
    import jax
    from jax.experimental.shard_map import shard_map
    from jax.sharding import Mesh, NamedSharding, PartitionSpec

    import concourse.mybir as mybir
    from concourse.bass2jax import (
        _bass_exec_p,
        install_neuronx_cc_hook,
        partition_id_tensor,
    )

    install_neuronx_cc_hook()

    in_names, out_names, out_avals = [], [], []
    pname = nc.partition_id_tensor.name if nc.partition_id_tensor else None
    for alloc in nc.m.functions[0].allocations:
        if not isinstance(alloc, mybir.MemoryLocationSet):
            continue
        name = alloc.memorylocations[0].name
        if alloc.kind == "ExternalInput":
            if name != pname:
                in_names.append(name)
        elif alloc.kind == "ExternalOutput":
            out_names.append(name)
            out_avals.append(jax.core.ShapedArray(
                tuple(alloc.tensor_shape), mybir.dt.np(alloc.dtype)))
    assert in_names == ["x"] and out_names == ["out"], (in_names, out_names)

    all_in = tuple(in_names) + tuple(out_names)
    if pname is not None:
        all_in = all_in + (pname,)
    bind_kwargs = dict(
        out_avals=tuple(out_avals),
        in_names=all_in,
        out_names=tuple(out_names),
        lowering_input_output_aliases=(),
        sim_require_finite=True,
        sim_require_nnan=True,
        nc=nc,
    )

    def _body(x, obuf):
        operands = [x, obuf]
        if pname is not None:
            operands.append(partition_id_tensor())
        (o,) = _bass_exec_p.bind(*operands, **bind_kwargs)
        return (o,)

    mesh = Mesh(np.asarray(jax.devices()[:N_CORES]), ("core",))
    fn = jax.jit(
        shard_map(_body, mesh=mesh,
                  in_specs=(PartitionSpec("core"), PartitionSpec("core")),
                  out_specs=(PartitionSpec("core"),),
                  check_rep=False),
        donate_argnums=(1,),
    )
    sharding = NamedSharding(mesh, PartitionSpec("core"))
    oshape = (N_CORES * out_avals[0].shape[0],) + tuple(out_avals[0].shape[1:])
    odtype = out_avals[0].dtype

    make_zeros = jax.jit(lambda: jax.numpy.zeros(oshape, odtype),
                         out_shardings=sharding)

    def run(x_concat: np.ndarray) -> np.ndarray:
        x_dev = jax.device_put(x_concat, sharding)
        (o,) = fn(x_dev, make_zeros())
        return np.asarray(o)

    return run


def _run_spmd_fallback(nc, x2: np.ndarray) -> np.ndarray:
    """Canonical path: bass_utils.run_bass_kernel_spmd (works both under
    axon/PJRT and with native /dev/neuron* NRT)."""
    from concourse.bass_utils import run_bass_kernel_spmd

    in_maps = [
        {"x": x2[c * TOK_PER_CORE:(c + 1) * TOK_PER_CORE]}
        for c in range(N_CORES)
    ]
    res = run_bass_kernel_spmd(nc, in_maps, core_ids=list(range(N_CORES)))
    return np.concatenate([r["out"] for r in res.results], axis=0)


# triangle layout tables (mirrors build_program's layout="tri")
TRI_LEN = [(i + 2) // 2 * 2 for i in range(D)]
TRI_OFF = [D + sum(TRI_LEN[:i]) for i in range(D)]


def _sym_index():
    """Device column holding ref sq element (i, j) in the tri layout."""
    idx = np.empty(D * D, np.int64)
    for f in range(D * D):
        i, j = f // D, f % D
        idx[f] = TRI_OFF[i] + j if j < TRI_LEN[i] else TRI_OFF[j] + i
    return idx


_SYM = _sym_index()

# kernel() build configuration (selected by measurement; see module docstring)
BEST_CFG = {
    "hbm_dt": "float16",
    "layout": "tri",
    "sq_mode": "pair",
    "sq_loop": "i16",     # ignored for layout="tri" (tri has its own loop)
    "op_bufs": 2,
    "ot_split": True,
    "ladder": [32, 64, 80, 80],
}


def kernel(x: np.ndarray) -> np.ndarray:
    x = np.ascontiguousarray(np.asarray(x, dtype=np.float32))
    assert x.shape == (B, H, S, D), x.shape

    if "nc" not in _CACHE:
        _CACHE["nc"] = build_program(**BEST_CFG)
        try:
            from concourse._compat import axon_active
            _CACHE["run"] = (_make_runner(_CACHE["nc"])
                             if axon_active() else None)
        except Exception:
            _CACHE["run"] = None

    # core c gets (b,h) slices [8c, 8c+8) -> concat over cores is just
    # the natural [BH*S, D] layout
    x2 = x.reshape(BH * S, D)
    out = None
    if _CACHE.get("run") is not None:
        try:
            out = _CACHE["run"](x2)      # cached fast path (axon/PJRT)
        except Exception:
            _CACHE["run"] = None
    if out is None:
        out = _run_spmd_fallback(_CACHE["nc"], x2)
    layout = BEST_CFG.get("layout", "ref")
    if layout in ("pad", "tri"):
        # device row = [x/rrd (16) | sq | 1 | pad]; reassemble the
        # reference column order on host — a pure gather/permutation +
        # f32 upcast of device-computed values ("tri" additionally
        # mirrors each symmetric pair from its single device copy)
        full = np.empty((BH * S, OUT_W), np.float32)
        full[:, 0] = out[:, -2]
        full[:, 1:1 + D] = out[:, 0:D]
        if layout == "tri":
            full[:, 1 + D:] = out[:, _SYM]
        else:
            full[:, 1 + D:] = out[:, D:D + D * D]
        out = full
    elif out.dtype != np.float32:
        out = np.asarray(out, dtype=np.float32)
    return out.reshape(B, H, S, OUT_W)



# revision 46
# speedup vs baseline: 1.0289x; 1.0257x over previous
"""Trainium2 Bass kernel for per-token quadratic feature map.

reference: x [B=4, H=16, S=4096, d=16] f32 ->
  out [B, H, S, 1 + d + d*d = 273] = concat([1, x/sqrt(sqrt(d)), (x_i*x_j)/(sqrt(2)*sqrt(d))])

Fully data-parallel per (b, h) slice: 64 slices sharded 8 per NeuronCore
across 8 cores (32768 tokens/core), no collectives.

The op is HBM-store-bound (per-NC HBM limit ~358 GB/s), so the kernel
minimizes device->HBM bytes while computing every unique output value on
device, at a precision far inside the 2e-2 tolerance gate:

* fp16 output rows (quantization rel err ~8e-4 vs the 2e-2 gate).
* symmetric compaction: x_i*x_j == x_j*x_i, so the device stores the
  lower triangle only (each row padded to even length), 144 of the 256
  products. Device row = [x/rrd (16) | tri (144) | 1 | pad] = 162 cols;
  the host gathers the full 273-column reference order from it (a pure
  permutation/duplication + f32 upcast of device-computed values -
  np.take with a constant index map, no arithmetic).
* the DVE outer product runs in packed 2x_1P mode (2 fp16/cycle): all
  tensor_tensor operands get innermost step +1 / 4B-aligned APs by
  reading y_i from a duplicated-pair tile (ydbl[t,2i]=ydbl[t,2i+1]=y_i,
  built on ScalarE) - a plain broadcast AP (step 0) would drop the DVE
  to 1x and make compute the bottleneck (measured 86 us that way).
* per 128-partition x nt-token tile: ScalarE builds x/rrd + y + ydbl,
  gpsimd memsets the ones column, DVE runs 16 ragged tensor_tensor ops
  (row i: j <= i), one HWDGE (SP ring) store per tile; loads ride the
  ACT ring. Ladder [32,64,80,80] tokens/partition with per-size output
  pools (double-buffered) overlaps compute with stores.

Per core: 2 MB in + 10.6 MB out = ~35 us DMA floor; measured slope
~40-47 us (HBM-neighbor dependent), ~2.6x the previous f32-output
version (~112 us), whose store stream alone needs ~101 us.
"""

import math

import numpy as np

B, H, S, D = 4, 16, 4096, 16
BH = B * H                      # 64 (b,h) slices
N_CORES = 8
SLICES_PER_CORE = BH // N_CORES  # 8
TOK_PER_CORE = SLICES_PER_CORE * S  # 32768
NT = 32                          # tokens per partition per tile
P = 128                          # partitions
TILE_TOK = P * NT                # 4096 tokens = one (b,h) slice
OUT_W = 1 + D + D * D            # 273

R2 = math.sqrt(2.0)
RD = math.sqrt(D)
RRD = math.sqrt(RD)
C_LIN = 1.0 / RRD                # linear-term scale
C_SQ = 1.0 / math.sqrt(R2 * RD)  # prescale: (x_i*C_SQ)*(x_j*C_SQ) = x_i*x_j/(R2*RD)
C_SQ2 = 1.0 / (R2 * RD)          # one-sided: (x_i*C_SQ2)*x_j = x_i*x_j/(R2*RD)

_CACHE = {}


def build_program(reps=1, loop_reps=0, ladder=None, op_bufs=4,
                 load_ring="scalar", prescale_eng="vector",
                 load_order="tile", decouple=False, ot_bf16=True,
                 ot_dt16="float16", ones_eng="gpsimd", hbm_dt="float32",
                 store_ring="sync", layout="ref", sq_mode="plain",
                 sq_loop=False, sq_gp_rows=0, xp_bufs=None, yp_bufs=6,
                 ot_split=False, ydbl_eng="scalar", merge01=False,
                 ones_once=False, yt_eng="scalar"):
    """Build + compile the per-core Bass program. `reps` statically repeats
    the whole pipeline; `loop_reps` wraps it in a hardware For_i loop (both
    used only for HW timing via slope). Non-default values of the remaining
    knobs exist for perf A/B only: `ot_bf16`/`ot_dt16` pick the 16-bit
    output-tile dtype (False = f32 tiles + HWDGE stores), `decouple` makes
    stores read a constant tile instead of the computed one."""
    from contextlib import ExitStack

    import concourse.bacc as bacc
    import concourse.mybir as mybir
    import concourse.tile as tile

    nc = bacc.Bacc("TRN2", target_bir_lowering=False, debug=False)
    hbm_dtype = getattr(mybir.dt, hbm_dt)
    # layout "ref":  row = [1 | x/rrd | sq], width 273 (reference order)
    # layout "pad":  row = [x/rrd | sq | 1 | pad], width 274 — keeps every
    #   fp16 (i, 2J) output pair 4B-aligned so the DVE runs packed 2x mode;
    #   host reorders columns (pure permutation, all values device-computed)
    # layout "tri":  row = [x/rrd | tri | 1 | pad], width 162 — sq is
    #   symmetric (y_i*y_j == y_j*y_i), so store only rows j<=i, each
    #   padded to even length for pair alignment; host mirrors the
    #   duplicate entries (pure gather of device-computed values)
    # layout "trix": exact triangle (136 cols, no even-padding): rows are
    #   reordered so even-length rows come first (all 4B-aligned starts ->
    #   DVE 2x); odd-length rows alternate long(aligned)/short(odd start,
    #   1x -- only lengths 1,3,5,7 pay it); odd-length rows write one
    #   stray element that the next row's op (same engine, in order)
    #   overwrites, so DRAM stays exactly 136 wide. Row i=15 last, clean.
    if layout == "tri":
        tri_len = [(i + 2) // 2 * 2 for i in range(D)]   # 2,2,4,4,...,16,16
        tri_off = [D + sum(tri_len[:i]) for i in range(D)]
        tri_order = list(range(D))
        OW = D + sum(tri_len) + 2                        # 162
    elif layout == "trix":
        tri_order = TRIX_ORDER
        tri_len = [i + 1 for i in range(D)]              # exact lengths
        tri_off = [0] * D
        cur = D
        for i in tri_order:
            tri_off[i] = cur
            cur += tri_len[i]
        OW = cur + 2                                     # 154
    elif layout == "pad":
        OW = OUT_W + 1
    else:
        OW = OUT_W
    x_d = nc.dram_tensor("x", [TOK_PER_CORE, D], mybir.dt.float32,
                         kind="ExternalInput")
    o_d = nc.dram_tensor("out", [TOK_PER_CORE, OW], hbm_dtype,
                         kind="ExternalOutput")

    # flat views: per tile, both input and output regions are contiguous
    x_flat = x_d.ap().rearrange("t d -> (t d)")
    o_flat = o_d.ap().rearrange("t d -> (t d)")

    # Tile-size ladder (tokens per partition per tile): small first tiles so
    # the first out-DMA launches early; 32-token (4.47 MB) tiles in steady
    # state, the probe-measured sweet spot for store throughput.
    if ladder is None:
        ladder = [4, 4, 8, 16] + [NT] * 7
    assert sum(ladder) == TOK_PER_CORE // P
    n_tiles = len(ladder)

    with tile.TileContext(nc) as tc, ExitStack() as ctx:
        xp = ctx.enter_context(tc.tile_pool(
            name="x", bufs=xp_bufs or n_tiles + 1))
        yp = ctx.enter_context(tc.tile_pool(name="y", bufs=yp_bufs))
        op = ctx.enter_context(tc.tile_pool(name="o", bufs=op_bufs))
        cst = None
        if decouple:
            # perf triage: stores read this constant tile instead of the
            # computed one, removing the compute->store dependency
            cp = ctx.enter_context(tc.tile_pool(name="c", bufs=1))
            cst = cp.tile([P, NT * OW], mybir.dt.float32,
                          tag="cst", name="cst")
            nc.gpsimd.memset(cst[:], 1.0)
        if ones_once:
            # the ones/pad columns of every output ring buffer are
            # constant 1.0 at a fixed offset -- initialize each slot once
            # before the loop instead of re-memsetting every tile
            dt16_pre = getattr(mybir.dt, hbm_dt if hbm_dt != "float32"
                               else ot_dt16)
            assert layout in ("pad", "tri", "trix")
            seen = {}
            for nt_ in ladder:
                tag = f"ot{nt_}" if ot_split else "ot"
                nbuf = op_bufs - seen.get(tag, 0)
                seen[tag] = op_bufs
                mx = nt_ if ot_split else max(ladder)
                for _ in range(nbuf):
                    pre = op.tile([P, mx * OW], dt16_pre, tag=tag,
                                  name="ot")
                    pre3 = pre[:].rearrange("p (t f) -> p t f", f=OW)
                    getattr(nc, ones_eng).memset(pre3[:, :, OW - 2:OW], 1.0)
        if loop_reps:
            ctx.enter_context(tc.For_i(0, loop_reps, 1))

        for _ in range(reps):
            xts, pos = [], 0
            if load_order == "front":
                # all input loads queued ahead of the stores on the same
                # ring (xt pool holds one slot per tile)
                for nt_ in ladder:
                    tile_tok = P * nt_
                    xt = xp.tile([P, nt_ * D], mybir.dt.float32, tag="xt",
                                 name="xt")
                    src = x_flat[pos * D:(pos + tile_tok) * D]
                    getattr(nc, load_ring).dma_start(
                        xt[:], src.rearrange("(p f) -> p f", p=P))
                    xts.append(xt)
                    pos += tile_tok
            else:
                xts = [None] * len(ladder)

            # per tile: (load if not front-loaded, then) compute + store
            pos = 0
            if hbm_dt != "float32":
                # 16-bit output straight to HBM: tiles must match hbm dtype
                ot_dt16 = hbm_dt
            dt16 = getattr(mybir.dt, ot_dt16)
            ot_dt = dt16 if ot_bf16 else mybir.dt.float32
            y_dt = dt16 if ot_bf16 else mybir.dt.float32
            for ti, (xt, nt_) in enumerate(zip(xts, ladder)):
                tile_tok = P * nt_
                if xt is None:
                    xt = xp.tile([P, nt_ * D], mybir.dt.float32, tag="xt",
                                 name="xt")
                    src = x_flat[pos * D:(pos + tile_tok) * D]
                    getattr(nc, load_ring).dma_start(
                        xt[:], src.rearrange("(p f) -> p f", p=P))
                yt = yp.tile([P, nt_ * D], y_dt, tag="yt", name="yt")
                ot_tag = f"ot{nt_}" if ot_split else "ot"
                ot = op.tile([P, nt_ * OW], ot_dt, tag=ot_tag, name="ot")

                ot3 = ot[:].rearrange("p (t f) -> p t f", f=OW)
                x3 = xt[:].rearrange("p (t f) -> p t f", f=D)

                if layout in ("pad", "tri", "trix"):
                    lin = ot3[:, :, 0:D]
                    sq_flat = ot3[:, :, D:OW - 2]
                    ones_sl = ot3[:, :, OW - 2:OW]  # ones + pad col
                else:
                    lin = ot3[:, :, 1:1 + D]
                    sq_flat = ot3[:, :, 1 + D:]
                    ones_sl = ot3[:, :, 0:1]

                # ones column (gpsimd by default so DVE/ACT stay free;
                # with ones_once the ring buffers were pre-initialized)
                if not ones_once:
                    getattr(nc, ones_eng).memset(ones_sl, 1.0)

                # linear term on ScalarE: x * C_LIN
                nc.scalar.mul(lin, x3, C_LIN)

                if sq_mode == "pair":
                    # packed-pair outer product: all DVE operands get
                    # innermost step +1 / 4B-aligned so tensor_tensor runs
                    # 2x_1P (2 fp16/cycle) instead of 1x. in0 reads from
                    # ydbl where each y_i appears twice consecutively.
                    ydbl = yp.tile([P, nt_ * 2 * D], y_dt, tag="ydbl",
                                   name="ydbl")
                    yd3 = ydbl[:].rearrange("p (t i pr) -> p t i pr",
                                            i=D, pr=2)
                    xdup = x3.unsqueeze(3).broadcast_to((P, nt_, D, 2))
                    if ydbl_eng == "scalar":
                        nc.scalar.mul(yd3, xdup, C_SQ)
                    else:
                        getattr(nc, ydbl_eng).tensor_scalar_mul(
                            yd3, xdup, C_SQ)
                    if yt_eng == "vector":
                        # yt on DVE (2x_2P tensor_scalar) in parallel with
                        # ACT's ydbl -- shortens the tile-0 critical chain
                        nc.vector.tensor_scalar_mul(yt[:], xt[:], C_SQ)
                    else:
                        nc.scalar.mul(yt[:], xt[:], C_SQ)
                    y4 = yt[:].rearrange("p (t J pr) -> p t J pr",
                                         J=D // 2, pr=2)
                    ndv = D - sq_gp_rows  # i-rows computed by DVE
                    if layout == "trix":
                        # exact triangle: rows in TRIX_ORDER; odd-length
                        # rows write one stray element overwritten by the
                        # next row's op (DVE is in-order, WAW-safe)
                        for i in tri_order:
                            J = (tri_len[i] + 1) // 2
                            out_i = (ot3[:, :,
                                         tri_off[i]:tri_off[i] + 2 * J]
                                     .rearrange("p t (J pr) -> p t J pr",
                                                pr=2))
                            in0 = (yd3[:, :, i:i + 1, :]
                                   .broadcast_to((P, nt_, J, 2)))
                            nc.vector.tensor_mul(out_i, in0,
                                                 y4[:, :, :J, :])
                    elif layout == "tri":
                        # ragged triangle: one op per i-row, j <= i
                        # (padded to even length for pair alignment)
                        i0 = 0
                        if merge01:
                            # rows 0 and 1 (len 2 each) fuse into one op:
                            # out run4 [y0*y0, y0*y1, y1*y0, y1*y1]
                            out01 = (ot3[:, :, tri_off[0]:tri_off[0] + 4]
                                     .rearrange("p t (i pr) -> p t i pr",
                                                pr=2))
                            in1 = (y4[:, :, 0:1, :]
                                   .broadcast_to((P, nt_, 2, 2)))
                            nc.vector.tensor_mul(out01, yd3[:, :, 0:2, :],
                                                 in1)
                            i0 = 2
                        for i in range(i0, D):
                            L = tri_len[i] // 2
                            out_i = (ot3[:, :,
                                         tri_off[i]:tri_off[i] + tri_len[i]]
                                     .rearrange("p t (J pr) -> p t J pr",
                                                pr=2))
                            in0 = (yd3[:, :, i:i + 1, :]
                                   .broadcast_to((P, nt_, L, 2)))
                            nc.vector.tensor_mul(out_i, in0,
                                                 y4[:, :, :L, :])
                    elif sq_loop == "j8":
                        # one op per output column-pair J: in0 = the whole
                        # ydbl tile (fully contiguous, no broadcast), in1 =
                        # the J-th y pair broadcast over i. All APs have
                        # innermost step +1 and 4B-aligned starts -> 2x_1P.
                        sq5 = sq_flat.rearrange(
                            "p t (i J pr) -> p t i J pr", J=D // 2, pr=2)
                        ngj = sq_gp_rows // 2  # J-ops on gpsimd (from top)
                        for Jf in range(D // 2):
                            eng = nc.vector if Jf < D // 2 - ngj \
                                else nc.gpsimd
                            in1 = (y4[:, :, Jf:Jf + 1, :]
                                   .broadcast_to((P, nt_, D, 2)))
                            eng.tensor_mul(sq5[:, :, :, Jf], yd3, in1)
                    elif sq_loop:
                        sq5 = sq_flat.rearrange(
                            "p t (i J pr) -> p t i J pr", J=D // 2, pr=2)
                        for i in range(D):
                            eng = nc.vector if i < ndv else nc.gpsimd
                            in0 = (yd3[:, :, i:i + 1, :]
                                   .broadcast_to((P, nt_, D // 2, 2)))
                            in1 = y4
                            eng.tensor_mul(sq5[:, :, i], in0, in1)
                    else:
                        sq5 = sq_flat.rearrange(
                            "p t (i J pr) -> p t i J pr", J=D // 2, pr=2)
                        in0 = (yd3.unsqueeze(3)
                               .broadcast_to((P, nt_, D, D // 2, 2)))
                        in1 = (y4.unsqueeze(2)
                               .broadcast_to((P, nt_, D, D // 2, 2)))
                        if sq_gp_rows:
                            nc.vector.tensor_mul(
                                sq5[:, :, :ndv], in0[:, :, :ndv],
                                in1[:, :, :ndv])
                            nc.gpsimd.tensor_mul(
                                sq5[:, :, ndv:], in0[:, :, ndv:],
                                in1[:, :, ndv:])
                        else:
                            nc.vector.tensor_mul(sq5, in0, in1)
                else:
                    # prescale y = x * C_SQ (ScalarE by default; DVE then
                    # runs exactly one op per tile, the big outer product)
                    getattr(nc, prescale_eng).mul(yt[:], xt[:], C_SQ) \
                        if prescale_eng == "scalar" else \
                        nc.vector.tensor_scalar_mul(yt[:], xt[:], C_SQ)

                    # outer products: broadcast-AP DVE tensor_tensor
                    y3 = yt[:].rearrange("p (t f) -> p t f", f=D)
                    sq = sq_flat.rearrange("p t (i j) -> p t i j", j=D)
                    ndv = D - sq_gp_rows
                    in0 = y3.unsqueeze(3).broadcast_to((P, nt_, D, D))
                    in1 = y3.unsqueeze(2).broadcast_to((P, nt_, D, D))
                    if sq_gp_rows:
                        nc.vector.tensor_mul(
                            sq[:, :, :ndv], in0[:, :, :ndv],
                            in1[:, :, :ndv])
                        nc.gpsimd.tensor_mul(
                            sq[:, :, ndv:], in0[:, :, ndv:],
                            in1[:, :, ndv:])
                    else:
                        nc.vector.tensor_mul(sq, in0, in1)

                # store: contiguous (up to 4.47 MB) on the SP ring. With a
                # bf16 output tile the store goes via SWDGE (gpsimd), which
                # upcasts bf16->f32 inline during the DMA; HBM still
                # receives the full f32 output.
                dst = o_flat[pos * OW:(pos + tile_tok) * OW]
                src_t = cst[:, :nt_ * OW] if decouple else ot[:]
                if ot_bf16 and hbm_dt == "float32":
                    # 16-bit tile, f32 HBM: SWDGE casts inline during DMA
                    nc.gpsimd.dma_start(
                        dst.rearrange("(p f) -> p f", p=P), src_t)
                else:
                    # dtypes match: plain HWDGE store
                    ring = store_ring
                    if ring == "alt":  # alternate SP / ACT HWDGE rings
                        ring = "sync" if ti % 2 == 0 else "scalar"
                    getattr(nc, ring).dma_start(
                        dst.rearrange("(p f) -> p f", p=P), src_t)
                pos += tile_tok

    nc.compile()
    return nc


def _make_runner(nc):
    """One-time: build a cached jitted shard_map executor for `nc`."""
    import jax
    from jax.experimental.shard_map import shard_map
    from jax.sharding import Mesh, NamedSharding, PartitionSpec

    import concourse.mybir as mybir
    from concourse.bass2jax import (
        _bass_exec_p,
        install_neuronx_cc_hook,
        partition_id_tensor,
    )

    install_neuronx_cc_hook()

    in_names, out_names, out_avals = [], [], []
    pname = nc.partition_id_tensor.name if nc.partition_id_tensor else None
    for alloc in nc.m.functions[0].allocations:
        if not isinstance(alloc, mybir.MemoryLocationSet):
            continue
        name = alloc.memorylocations[0].name
        if alloc.kind == "ExternalInput":
            if name != pname:
                in_names.append(name)
        elif alloc.kind == "ExternalOutput":
            out_names.append(name)
            out_avals.append(jax.core.ShapedArray(
                tuple(alloc.tensor_shape), mybir.dt.np(alloc.dtype)))
    assert in_names == ["x"] and out_names == ["out"], (in_names, out_names)

    all_in = tuple(in_names) + tuple(out_names)
    if pname is not None:
        all_in = all_in + (pname,)
    bind_kwargs = dict(
        out_avals=tuple(out_avals),
        in_names=all_in,
        out_names=tuple(out_names),
        lowering_input_output_aliases=(),
        sim_require_finite=True,
        sim_require_nnan=True,
        nc=nc,
    )

    def _body(x, obuf):
        operands = [x, obuf]
        if pname is not None:
            operands.append(partition_id_tensor())
        (o,) = _bass_exec_p.bind(*operands, **bind_kwargs)
        return (o,)

    mesh = Mesh(np.asarray(jax.devices()[:N_CORES]), ("core",))
    fn = jax.jit(
        shard_map(_body, mesh=mesh,
                  in_specs=(PartitionSpec("core"), PartitionSpec("core")),
                  out_specs=(PartitionSpec("core"),),
                  check_rep=False),
        donate_argnums=(1,),
    )
    sharding = NamedSharding(mesh, PartitionSpec("core"))
    oshape = (N_CORES * out_avals[0].shape[0],) + tuple(out_avals[0].shape[1:])
    odtype = out_avals[0].dtype

    make_zeros = jax.jit(lambda: jax.numpy.zeros(oshape, odtype),
                         out_shardings=sharding)

    def run(x_concat: np.ndarray) -> np.ndarray:
        x_dev = jax.device_put(x_concat, sharding)
        (o,) = fn(x_dev, make_zeros())
        return np.asarray(o)

    return run


def _run_spmd_fallback(nc, x2: np.ndarray) -> np.ndarray:
    """Canonical path: bass_utils.run_bass_kernel_spmd (works both under
    axon/PJRT and with native /dev/neuron* NRT)."""
    from concourse.bass_utils import run_bass_kernel_spmd

    in_maps = [
        {"x": x2[c * TOK_PER_CORE:(c + 1) * TOK_PER_CORE]}
        for c in range(N_CORES)
    ]
    res = run_bass_kernel_spmd(nc, in_maps, core_ids=list(range(N_CORES)))
    return np.concatenate([r["out"] for r in res.results], axis=0)


# triangle layout tables (mirrors build_program's layout="tri")
TRI_LEN = [(i + 2) // 2 * 2 for i in range(D)]
TRI_OFF = [D + sum(TRI_LEN[:i]) for i in range(D)]

# exact-triangle ("trix") row order: even-length rows first, then
# odd-length rows long/short interleaved, i=15 last (see build_program)
TRIX_ORDER = [1, 3, 5, 7, 9, 11, 13] + [14, 0, 12, 2, 10, 4, 8, 6] + [15]


TRIX_OFF = [0] * D
_cur = D
for _i in TRIX_ORDER:
    TRIX_OFF[_i] = _cur
    _cur += _i + 1


def _sym_index(off, length):
    """Device column holding ref sq element (i, j) in a triangle layout."""
    idx = np.empty(D * D, np.int64)
    for f in range(D * D):
        i, j = f // D, f % D
        idx[f] = off[i] + j if j < length[i] else off[j] + i
    return idx


_SYM = _sym_index(TRI_OFF, TRI_LEN)
_SYM_X = _sym_index(TRIX_OFF, [i + 1 for i in range(D)])

# kernel() build configuration (selected by measurement; see module docstring)
BEST_CFG = {
    "hbm_dt": "float16",
    "layout": "tri",
    "sq_mode": "pair",
    "sq_loop": "i16",     # ignored for layout="tri" (tri has its own loop)
    "op_bufs": 2,
    "ot_split": True,
    "ladder": [32, 64, 80, 80],
}


def kernel(x: np.ndarray) -> np.ndarray:
    x = np.ascontiguousarray(np.asarray(x, dtype=np.float32))
    assert x.shape == (B, H, S, D), x.shape

    if "nc" not in _CACHE:
        _CACHE["nc"] = build_program(**BEST_CFG)
        try:
            from concourse._compat import axon_active
            _CACHE["run"] = (_make_runner(_CACHE["nc"])
                             if axon_active() else None)
        except Exception:
            _CACHE["run"] = None

    # core c gets (b,h) slices [8c, 8c+8) -> concat over cores is just
    # the natural [BH*S, D] layout
    x2 = x.reshape(BH * S, D)
    out = None
    if _CACHE.get("run") is not None:
        try:
            out = _CACHE["run"](x2)      # cached fast path (axon/PJRT)
        except Exception:
            _CACHE["run"] = None
    if out is None:
        out = _run_spmd_fallback(_CACHE["nc"], x2)
    layout = BEST_CFG.get("layout", "ref")
    if layout in ("pad", "tri", "trix"):
        # device row = [x/rrd (16) | sq | 1 | pad]; reassemble the
        # reference column order on host — a pure gather/permutation +
        # f32 upcast of device-computed values ("tri" additionally
        # mirrors each symmetric pair from its single device copy)
        full = np.empty((BH * S, OUT_W), np.float32)
        full[:, 0] = out[:, -2]
        full[:, 1:1 + D] = out[:, 0:D]
        if layout == "tri":
            full[:, 1 + D:] = out[:, _SYM]
        elif layout == "trix":
            full[:, 1 + D:] = out[:, _SYM_X]
        else:
            full[:, 1 + D:] = out[:, D:D + D * D]
        out = full
    elif out.dtype != np.float32:
        out = np.asarray(out, dtype=np.float32)
    return out.reshape(B, H, S, OUT_W)



# revision 48
# speedup vs baseline: 1.0416x; 1.0123x over previous
"""Trainium2 Bass kernel for per-token quadratic feature map.

reference: x [B=4, H=16, S=4096, d=16] f32 ->
  out [B, H, S, 1 + d + d*d = 273] = concat([1, x/sqrt(sqrt(d)), (x_i*x_j)/(sqrt(2)*sqrt(d))])

Fully data-parallel per (b, h) slice: 64 slices sharded 8 per NeuronCore
across 8 cores (32768 tokens/core), no collectives.

The op is HBM-store-bound (per-NC HBM limit ~358 GB/s), so the kernel
minimizes device->HBM bytes while computing every unique output value on
device, at a precision far inside the 2e-2 tolerance gate:

* fp16 output rows (quantization rel err ~8e-4 vs the 2e-2 gate).
* symmetric compaction: x_i*x_j == x_j*x_i, so the device stores the
  lower triangle only (each row padded to even length), 144 of the 256
  products. Device row = [x/rrd (16) | tri (144) | 1 | pad] = 162 cols;
  the host gathers the full 273-column reference order from it (a pure
  permutation/duplication + f32 upcast of device-computed values -
  np.take with a constant index map, no arithmetic).
* the DVE outer product runs in packed 2x_1P mode (2 fp16/cycle): all
  tensor_tensor operands get innermost step +1 / 4B-aligned APs by
  reading y_i from a duplicated-pair tile (ydbl[t,2i]=ydbl[t,2i+1]=y_i,
  built on ScalarE) - a plain broadcast AP (step 0) would drop the DVE
  to 1x and make compute the bottleneck (measured 86 us that way).
* per 128-partition x nt-token tile: ScalarE builds x/rrd + y + ydbl,
  gpsimd memsets the ones column, DVE runs 16 ragged tensor_tensor ops
  (row i: j <= i), one HWDGE (SP ring) store per tile; loads ride the
  ACT ring. Ladder [32,56,56,56,56] tokens/partition with per-size
  triple-buffered output pools overlaps compute with stores.

Per core: 2 MB in + 10.6 MB out = ~35 us DMA floor; measured slope
~40-47 us (HBM-neighbor dependent), ~2.6x the previous f32-output
version (~112 us), whose store stream alone needs ~101 us.
"""

import math

import numpy as np

B, H, S, D = 4, 16, 4096, 16
BH = B * H                      # 64 (b,h) slices
N_CORES = 8
SLICES_PER_CORE = BH // N_CORES  # 8
TOK_PER_CORE = SLICES_PER_CORE * S  # 32768
NT = 32                          # tokens per partition per tile
P = 128                          # partitions
TILE_TOK = P * NT                # 4096 tokens = one (b,h) slice
OUT_W = 1 + D + D * D            # 273

R2 = math.sqrt(2.0)
RD = math.sqrt(D)
RRD = math.sqrt(RD)
C_LIN = 1.0 / RRD                # linear-term scale
C_SQ = 1.0 / math.sqrt(R2 * RD)  # prescale: (x_i*C_SQ)*(x_j*C_SQ) = x_i*x_j/(R2*RD)
C_SQ2 = 1.0 / (R2 * RD)          # one-sided: (x_i*C_SQ2)*x_j = x_i*x_j/(R2*RD)

_CACHE = {}


def build_program(reps=1, loop_reps=0, ladder=None, op_bufs=4,
                 load_ring="scalar", prescale_eng="vector",
                 load_order="tile", decouple=False, ot_bf16=True,
                 ot_dt16="float16", ones_eng="gpsimd", hbm_dt="float32",
                 store_ring="sync", layout="ref", sq_mode="plain",
                 sq_loop=False, sq_gp_rows=0, xp_bufs=None, yp_bufs=6,
                 ot_split=False, ydbl_eng="scalar", merge01=False,
                 ones_once=False, yt_eng="scalar"):
    """Build + compile the per-core Bass program. `reps` statically repeats
    the whole pipeline; `loop_reps` wraps it in a hardware For_i loop (both
    used only for HW timing via slope). Non-default values of the remaining
    knobs exist for perf A/B only: `ot_bf16`/`ot_dt16` pick the 16-bit
    output-tile dtype (False = f32 tiles + HWDGE stores), `decouple` makes
    stores read a constant tile instead of the computed one."""
    from contextlib import ExitStack

    import concourse.bacc as bacc
    import concourse.mybir as mybir
    import concourse.tile as tile

    nc = bacc.Bacc("TRN2", target_bir_lowering=False, debug=False)
    hbm_dtype = getattr(mybir.dt, hbm_dt)
    # layout "ref":  row = [1 | x/rrd | sq], width 273 (reference order)
    # layout "pad":  row = [x/rrd | sq | 1 | pad], width 274 — keeps every
    #   fp16 (i, 2J) output pair 4B-aligned so the DVE runs packed 2x mode;
    #   host reorders columns (pure permutation, all values device-computed)
    # layout "tri":  row = [x/rrd | tri | 1 | pad], width 162 — sq is
    #   symmetric (y_i*y_j == y_j*y_i), so store only rows j<=i, each
    #   padded to even length for pair alignment; host mirrors the
    #   duplicate entries (pure gather of device-computed values)
    # layout "trix": exact triangle (136 cols, no even-padding): rows are
    #   reordered so even-length rows come first (all 4B-aligned starts ->
    #   DVE 2x); odd-length rows alternate long(aligned)/short(odd start,
    #   1x -- only lengths 1,3,5,7 pay it); odd-length rows write one
    #   stray element that the next row's op (same engine, in order)
    #   overwrites, so DRAM stays exactly 136 wide. Row i=15 last, clean.
    if layout == "tri":
        tri_len = [(i + 2) // 2 * 2 for i in range(D)]   # 2,2,4,4,...,16,16
        tri_off = [D + sum(tri_len[:i]) for i in range(D)]
        tri_order = list(range(D))
        OW = D + sum(tri_len) + 2                        # 162
    elif layout == "trix":
        tri_order = TRIX_ORDER
        tri_len = [i + 1 for i in range(D)]              # exact lengths
        tri_off = [0] * D
        cur = D
        for i in tri_order:
            tri_off[i] = cur
            cur += tri_len[i]
        OW = cur + 2                                     # 154
    elif layout == "pad":
        OW = OUT_W + 1
    else:
        OW = OUT_W
    x_d = nc.dram_tensor("x", [TOK_PER_CORE, D], mybir.dt.float32,
                         kind="ExternalInput")
    o_d = nc.dram_tensor("out", [TOK_PER_CORE, OW], hbm_dtype,
                         kind="ExternalOutput")

    # flat views: per tile, both input and output regions are contiguous
    x_flat = x_d.ap().rearrange("t d -> (t d)")
    o_flat = o_d.ap().rearrange("t d -> (t d)")

    # Tile-size ladder (tokens per partition per tile): small first tiles so
    # the first out-DMA launches early; 32-token (4.47 MB) tiles in steady
    # state, the probe-measured sweet spot for store throughput.
    if ladder is None:
        ladder = [4, 4, 8, 16] + [NT] * 7
    assert sum(ladder) == TOK_PER_CORE // P
    n_tiles = len(ladder)

    with tile.TileContext(nc) as tc, ExitStack() as ctx:
        xp = ctx.enter_context(tc.tile_pool(
            name="x", bufs=xp_bufs or n_tiles + 1))
        yp = ctx.enter_context(tc.tile_pool(name="y", bufs=yp_bufs))
        op = ctx.enter_context(tc.tile_pool(name="o", bufs=op_bufs))
        cst = None
        if decouple:
            # perf triage: stores read this constant tile instead of the
            # computed one, removing the compute->store dependency
            cp = ctx.enter_context(tc.tile_pool(name="c", bufs=1))
            cst = cp.tile([P, NT * OW], mybir.dt.float32,
                          tag="cst", name="cst")
            nc.gpsimd.memset(cst[:], 1.0)
        if ones_once:
            # the ones/pad columns of every output ring buffer are
            # constant 1.0 at a fixed offset -- initialize each slot once
            # before the loop instead of re-memsetting every tile
            dt16_pre = getattr(mybir.dt, hbm_dt if hbm_dt != "float32"
                               else ot_dt16)
            assert layout in ("pad", "tri", "trix")
            seen = {}
            for nt_ in ladder:
                tag = f"ot{nt_}" if ot_split else "ot"
                nbuf = op_bufs - seen.get(tag, 0)
                seen[tag] = op_bufs
                mx = nt_ if ot_split else max(ladder)
                for _ in range(nbuf):
                    pre = op.tile([P, mx * OW], dt16_pre, tag=tag,
                                  name="ot")
                    pre3 = pre[:].rearrange("p (t f) -> p t f", f=OW)
                    getattr(nc, ones_eng).memset(pre3[:, :, OW - 2:OW], 1.0)
        if loop_reps:
            ctx.enter_context(tc.For_i(0, loop_reps, 1))

        for _ in range(reps):
            xts, pos = [], 0
            if load_order == "front":
                # all input loads queued ahead of the stores on the same
                # ring (xt pool holds one slot per tile)
                for nt_ in ladder:
                    tile_tok = P * nt_
                    xt = xp.tile([P, nt_ * D], mybir.dt.float32, tag="xt",
                                 name="xt")
                    src = x_flat[pos * D:(pos + tile_tok) * D]
                    getattr(nc, load_ring).dma_start(
                        xt[:], src.rearrange("(p f) -> p f", p=P))
                    xts.append(xt)
                    pos += tile_tok
            else:
                xts = [None] * len(ladder)

            # per tile: (load if not front-loaded, then) compute + store
            pos = 0
            if hbm_dt != "float32":
                # 16-bit output straight to HBM: tiles must match hbm dtype
                ot_dt16 = hbm_dt
            dt16 = getattr(mybir.dt, ot_dt16)
            ot_dt = dt16 if ot_bf16 else mybir.dt.float32
            y_dt = dt16 if ot_bf16 else mybir.dt.float32
            for ti, (xt, nt_) in enumerate(zip(xts, ladder)):
                tile_tok = P * nt_
                if xt is None:
                    xt = xp.tile([P, nt_ * D], mybir.dt.float32, tag="xt",
                                 name="xt")
                    src = x_flat[pos * D:(pos + tile_tok) * D]
                    getattr(nc, load_ring).dma_start(
                        xt[:], src.rearrange("(p f) -> p f", p=P))
                yt = yp.tile([P, nt_ * D], y_dt, tag="yt", name="yt")
                ot_tag = f"ot{nt_}" if ot_split else "ot"
                ot = op.tile([P, nt_ * OW], ot_dt, tag=ot_tag, name="ot")

                ot3 = ot[:].rearrange("p (t f) -> p t f", f=OW)
                x3 = xt[:].rearrange("p (t f) -> p t f", f=D)

                if layout in ("pad", "tri", "trix"):
                    lin = ot3[:, :, 0:D]
                    sq_flat = ot3[:, :, D:OW - 2]
                    ones_sl = ot3[:, :, OW - 2:OW]  # ones + pad col
                else:
                    lin = ot3[:, :, 1:1 + D]
                    sq_flat = ot3[:, :, 1 + D:]
                    ones_sl = ot3[:, :, 0:1]

                # ones column (gpsimd by default so DVE/ACT stay free;
                # with ones_once the ring buffers were pre-initialized)
                if not ones_once:
                    getattr(nc, ones_eng).memset(ones_sl, 1.0)

                # linear term on ScalarE: x * C_LIN
                nc.scalar.mul(lin, x3, C_LIN)

                if sq_mode == "pair":
                    # packed-pair outer product: all DVE operands get
                    # innermost step +1 / 4B-aligned so tensor_tensor runs
                    # 2x_1P (2 fp16/cycle) instead of 1x. in0 reads from
                    # ydbl where each y_i appears twice consecutively.
                    ydbl = yp.tile([P, nt_ * 2 * D], y_dt, tag="ydbl",
                                   name="ydbl")
                    yd3 = ydbl[:].rearrange("p (t i pr) -> p t i pr",
                                            i=D, pr=2)
                    xdup = x3.unsqueeze(3).broadcast_to((P, nt_, D, 2))
                    if ydbl_eng == "scalar":
                        nc.scalar.mul(yd3, xdup, C_SQ)
                    else:
                        getattr(nc, ydbl_eng).tensor_scalar_mul(
                            yd3, xdup, C_SQ)
                    if yt_eng == "vector":
                        # yt on DVE (2x_2P tensor_scalar) in parallel with
                        # ACT's ydbl -- shortens the tile-0 critical chain
                        nc.vector.tensor_scalar_mul(yt[:], xt[:], C_SQ)
                    else:
                        nc.scalar.mul(yt[:], xt[:], C_SQ)
                    y4 = yt[:].rearrange("p (t J pr) -> p t J pr",
                                         J=D // 2, pr=2)
                    ndv = D - sq_gp_rows  # i-rows computed by DVE
                    if layout == "trix":
                        # exact triangle: rows in TRIX_ORDER; odd-length
                        # rows write one stray element overwritten by the
                        # next row's op (DVE is in-order, WAW-safe)
                        for i in tri_order:
                            J = (tri_len[i] + 1) // 2
                            out_i = (ot3[:, :,
                                         tri_off[i]:tri_off[i] + 2 * J]
                                     .rearrange("p t (J pr) -> p t J pr",
                                                pr=2))
                            in0 = (yd3[:, :, i:i + 1, :]
                                   .broadcast_to((P, nt_, J, 2)))
                            nc.vector.tensor_mul(out_i, in0,
                                                 y4[:, :, :J, :])
                    elif layout == "tri":
                        # ragged triangle: one op per i-row, j <= i
                        # (padded to even length for pair alignment)
                        i0 = 0
                        if merge01:
                            # rows 0 and 1 (len 2 each) fuse into one op:
                            # out run4 [y0*y0, y0*y1, y1*y0, y1*y1]
                            out01 = (ot3[:, :, tri_off[0]:tri_off[0] + 4]
                                     .rearrange("p t (i pr) -> p t i pr",
                                                pr=2))
                            in1 = (y4[:, :, 0:1, :]
                                   .broadcast_to((P, nt_, 2, 2)))
                            nc.vector.tensor_mul(out01, yd3[:, :, 0:2, :],
                                                 in1)
                            i0 = 2
                        for i in range(i0, D):
                            L = tri_len[i] // 2
                            out_i = (ot3[:, :,
                                         tri_off[i]:tri_off[i] + tri_len[i]]
                                     .rearrange("p t (J pr) -> p t J pr",
                                                pr=2))
                            in0 = (yd3[:, :, i:i + 1, :]
                                   .broadcast_to((P, nt_, L, 2)))
                            nc.vector.tensor_mul(out_i, in0,
                                                 y4[:, :, :L, :])
                    elif sq_loop == "j8":
                        # one op per output column-pair J: in0 = the whole
                        # ydbl tile (fully contiguous, no broadcast), in1 =
                        # the J-th y pair broadcast over i. All APs have
                        # innermost step +1 and 4B-aligned starts -> 2x_1P.
                        sq5 = sq_flat.rearrange(
                            "p t (i J pr) -> p t i J pr", J=D // 2, pr=2)
                        ngj = sq_gp_rows // 2  # J-ops on gpsimd (from top)
                        for Jf in range(D // 2):
                            eng = nc.vector if Jf < D // 2 - ngj \
                                else nc.gpsimd
                            in1 = (y4[:, :, Jf:Jf + 1, :]
                                   .broadcast_to((P, nt_, D, 2)))
                            eng.tensor_mul(sq5[:, :, :, Jf], yd3, in1)
                    elif sq_loop:
                        sq5 = sq_flat.rearrange(
                            "p t (i J pr) -> p t i J pr", J=D // 2, pr=2)
                        for i in range(D):
                            eng = nc.vector if i < ndv else nc.gpsimd
                            in0 = (yd3[:, :, i:i + 1, :]
                                   .broadcast_to((P, nt_, D // 2, 2)))
                            in1 = y4
                            eng.tensor_mul(sq5[:, :, i], in0, in1)
                    else:
                        sq5 = sq_flat.rearrange(
                            "p t (i J pr) -> p t i J pr", J=D // 2, pr=2)
                        in0 = (yd3.unsqueeze(3)
                               .broadcast_to((P, nt_, D, D // 2, 2)))
                        in1 = (y4.unsqueeze(2)
                               .broadcast_to((P, nt_, D, D // 2, 2)))
                        if sq_gp_rows:
                            nc.vector.tensor_mul(
                                sq5[:, :, :ndv], in0[:, :, :ndv],
                                in1[:, :, :ndv])
                            nc.gpsimd.tensor_mul(
                                sq5[:, :, ndv:], in0[:, :, ndv:],
                                in1[:, :, ndv:])
                        else:
                            nc.vector.tensor_mul(sq5, in0, in1)
                else:
                    # prescale y = x * C_SQ (ScalarE by default; DVE then
                    # runs exactly one op per tile, the big outer product)
                    getattr(nc, prescale_eng).mul(yt[:], xt[:], C_SQ) \
                        if prescale_eng == "scalar" else \
                        nc.vector.tensor_scalar_mul(yt[:], xt[:], C_SQ)

                    # outer products: broadcast-AP DVE tensor_tensor
                    y3 = yt[:].rearrange("p (t f) -> p t f", f=D)
                    sq = sq_flat.rearrange("p t (i j) -> p t i j", j=D)
                    ndv = D - sq_gp_rows
                    in0 = y3.unsqueeze(3).broadcast_to((P, nt_, D, D))
                    in1 = y3.unsqueeze(2).broadcast_to((P, nt_, D, D))
                    if sq_gp_rows:
                        nc.vector.tensor_mul(
                            sq[:, :, :ndv], in0[:, :, :ndv],
                            in1[:, :, :ndv])
                        nc.gpsimd.tensor_mul(
                            sq[:, :, ndv:], in0[:, :, ndv:],
                            in1[:, :, ndv:])
                    else:
                        nc.vector.tensor_mul(sq, in0, in1)

                # store: contiguous (up to 4.47 MB) on the SP ring. With a
                # bf16 output tile the store goes via SWDGE (gpsimd), which
                # upcasts bf16->f32 inline during the DMA; HBM still
                # receives the full f32 output.
                dst = o_flat[pos * OW:(pos + tile_tok) * OW]
                src_t = cst[:, :nt_ * OW] if decouple else ot[:]
                if ot_bf16 and hbm_dt == "float32":
                    # 16-bit tile, f32 HBM: SWDGE casts inline during DMA
                    nc.gpsimd.dma_start(
                        dst.rearrange("(p f) -> p f", p=P), src_t)
                else:
                    # dtypes match: plain HWDGE store
                    ring = store_ring
                    if ring == "alt":  # alternate SP / ACT HWDGE rings
                        ring = "sync" if ti % 2 == 0 else "scalar"
                    getattr(nc, ring).dma_start(
                        dst.rearrange("(p f) -> p f", p=P), src_t)
                pos += tile_tok

    nc.compile()
    return nc


def _make_runner(nc):
    """One-time: build a cached jitted shard_map executor for `nc`."""
    import jax
    from jax.experimental.shard_map import shard_map
    from jax.sharding import Mesh, NamedSharding, PartitionSpec

    import concourse.mybir as mybir
    from concourse.bass2jax import (
        _bass_exec_p,
        install_neuronx_cc_hook,
        partition_id_tensor,
    )

    install_neuronx_cc_hook()

    in_names, out_names, out_avals = [], [], []
    pname = nc.partition_id_tensor.name if nc.partition_id_tensor else None
    for alloc in nc.m.functions[0].allocations:
        if not isinstance(alloc, mybir.MemoryLocationSet):
            continue
        name = alloc.memorylocations[0].name
        if alloc.kind == "ExternalInput":
            if name != pname:
                in_names.append(name)
        elif alloc.kind == "ExternalOutput":
            out_names.append(name)
            out_avals.append(jax.core.ShapedArray(
                tuple(alloc.tensor_shape), mybir.dt.np(alloc.dtype)))
    assert in_names == ["x"] and out_names == ["out"], (in_names, out_names)

    all_in = tuple(in_names) + tuple(out_names)
    if pname is not None:
        all_in = all_in + (pname,)
    bind_kwargs = dict(
        out_avals=tuple(out_avals),
        in_names=all_in,
        out_names=tuple(out_names),
        lowering_input_output_aliases=(),
        sim_require_finite=True,
        sim_require_nnan=True,
        nc=nc,
    )

    def _body(x, obuf):
        operands = [x, obuf]
        if pname is not None:
            operands.append(partition_id_tensor())
        (o,) = _bass_exec_p.bind(*operands, **bind_kwargs)
        return (o,)

    mesh = Mesh(np.asarray(jax.devices()[:N_CORES]), ("core",))
    fn = jax.jit(
        shard_map(_body, mesh=mesh,
                  in_specs=(PartitionSpec("core"), PartitionSpec("core")),
                  out_specs=(PartitionSpec("core"),),
                  check_rep=False),
        donate_argnums=(1,),
    )
    sharding = NamedSharding(mesh, PartitionSpec("core"))
    oshape = (N_CORES * out_avals[0].shape[0],) + tuple(out_avals[0].shape[1:])
    odtype = out_avals[0].dtype

    make_zeros = jax.jit(lambda: jax.numpy.zeros(oshape, odtype),
                         out_shardings=sharding)

    def run(x_concat: np.ndarray) -> np.ndarray:
        x_dev = jax.device_put(x_concat, sharding)
        (o,) = fn(x_dev, make_zeros())
        return np.asarray(o)

    return run


def _run_spmd_fallback(nc, x2: np.ndarray) -> np.ndarray:
    """Canonical path: bass_utils.run_bass_kernel_spmd (works both under
    axon/PJRT and with native /dev/neuron* NRT)."""
    from concourse.bass_utils import run_bass_kernel_spmd

    in_maps = [
        {"x": x2[c * TOK_PER_CORE:(c + 1) * TOK_PER_CORE]}
        for c in range(N_CORES)
    ]
    res = run_bass_kernel_spmd(nc, in_maps, core_ids=list(range(N_CORES)))
    return np.concatenate([r["out"] for r in res.results], axis=0)


# triangle layout tables (mirrors build_program's layout="tri")
TRI_LEN = [(i + 2) // 2 * 2 for i in range(D)]
TRI_OFF = [D + sum(TRI_LEN[:i]) for i in range(D)]

# exact-triangle ("trix") row order: even-length rows first, then
# odd-length rows long/short interleaved, i=15 last (see build_program)
TRIX_ORDER = [1, 3, 5, 7, 9, 11, 13] + [14, 0, 12, 2, 10, 4, 8, 6] + [15]


TRIX_OFF = [0] * D
_cur = D
for _i in TRIX_ORDER:
    TRIX_OFF[_i] = _cur
    _cur += _i + 1


def _sym_index(off, length):
    """Device column holding ref sq element (i, j) in a triangle layout."""
    idx = np.empty(D * D, np.int64)
    for f in range(D * D):
        i, j = f // D, f % D
        idx[f] = off[i] + j if j < length[i] else off[j] + i
    return idx


_SYM = _sym_index(TRI_OFF, TRI_LEN)
_SYM_X = _sym_index(TRIX_OFF, [i + 1 for i in range(D)])

# kernel() build configuration (selected by measurement; see module docstring)
BEST_CFG = {
    "hbm_dt": "float16",
    "layout": "tri",
    "sq_mode": "pair",
    "sq_loop": "i16",     # ignored for layout="tri" (tri has its own loop)
    "op_bufs": 3,
    "ot_split": True,
    "ladder": [32, 56, 56, 56, 56],
}


def kernel(x: np.ndarray) -> np.ndarray:
    x = np.ascontiguousarray(np.asarray(x, dtype=np.float32))
    assert x.shape == (B, H, S, D), x.shape

    if "nc" not in _CACHE:
        _CACHE["nc"] = build_program(**BEST_CFG)
        try:
            from concourse._compat import axon_active
            _CACHE["run"] = (_make_runner(_CACHE["nc"])
                             if axon_active() else None)
        except Exception:
            _CACHE["run"] = None

    # core c gets (b,h) slices [8c, 8c+8) -> concat over cores is just
    # the natural [BH*S, D] layout
    x2 = x.reshape(BH * S, D)
    out = None
    if _CACHE.get("run") is not None:
        try:
            out = _CACHE["run"](x2)      # cached fast path (axon/PJRT)
        except Exception:
            _CACHE["run"] = None
    if out is None:
        out = _run_spmd_fallback(_CACHE["nc"], x2)
    layout = BEST_CFG.get("layout", "ref")
    if layout in ("pad", "tri", "trix"):
        # device row = [x/rrd (16) | sq | 1 | pad]; reassemble the
        # reference column order on host — a pure gather/permutation +
        # f32 upcast of device-computed values ("tri" additionally
        # mirrors each symmetric pair from its single device copy)
        full = np.empty((BH * S, OUT_W), np.float32)
        full[:, 0] = out[:, -2]
        full[:, 1:1 + D] = out[:, 0:D]
        if layout == "tri":
            full[:, 1 + D:] = out[:, _SYM]
        elif layout == "trix":
            full[:, 1 + D:] = out[:, _SYM_X]
        else:
            full[:, 1 + D:] = out[:, D:D + D * D]
        out = full
    elif out.dtype != np.float32:
        out = np.asarray(out, dtype=np.float32)
    return out.reshape(B, H, S, OUT_W)



# revision 50
# speedup vs baseline: 1.0419x; 1.0003x over previous
"""Trainium2 Bass kernel for per-token quadratic feature map.

reference: x [B=4, H=16, S=4096, d=16] f32 ->
  out [B, H, S, 1 + d + d*d = 273] = concat([1, x/sqrt(sqrt(d)), (x_i*x_j)/(sqrt(2)*sqrt(d))])

Fully data-parallel per (b, h) slice: 64 slices sharded 8 per NeuronCore
across 8 cores (32768 tokens/core), no collectives.

The op is HBM-store-bound (per-NC HBM limit ~358 GB/s), so the kernel
minimizes device->HBM bytes while computing every unique output value on
device, at a precision far inside the 2e-2 tolerance gate:

* fp16 output rows (quantization rel err ~8e-4 vs the 2e-2 gate).
* symmetric compaction: x_i*x_j == x_j*x_i, so the device stores the
  lower triangle only (each row padded to even length), 144 of the 256
  products. Device row = [x/rrd (16) | tri (144) | 1 | pad] = 162 cols;
  the host gathers the full 273-column reference order from it (a pure
  permutation/duplication + f32 upcast of device-computed values -
  np.take with a constant index map, no arithmetic).
* the DVE outer product runs in packed 2x_1P mode (2 fp16/cycle): all
  tensor_tensor operands get innermost step +1 / 4B-aligned APs by
  reading y_i from a duplicated-pair tile (ydbl[t,2i]=ydbl[t,2i+1]=y_i,
  built on ScalarE) - a plain broadcast AP (step 0) would drop the DVE
  to 1x and make compute the bottleneck (measured 86 us that way).
* per 128-partition x nt-token tile: ScalarE builds x/rrd + y + ydbl,
  gpsimd memsets the ones column, DVE runs 16 ragged tensor_tensor ops
  (row i: j <= i), one HWDGE (SP ring) store per tile; loads ride the
  ACT ring. Ladder [32,48,48,48,48,32] tokens/partition (small last
  tile shortens the tail before the loop barrier) with per-size
  triple-buffered output pools overlaps compute with stores.

Per core: 2 MB in + 10.6 MB out = ~35 us DMA floor; measured slope
~40-47 us (HBM-neighbor dependent), ~2.6x the previous f32-output
version (~112 us), whose store stream alone needs ~101 us.
"""

import math

import numpy as np

B, H, S, D = 4, 16, 4096, 16
BH = B * H                      # 64 (b,h) slices
N_CORES = 8
SLICES_PER_CORE = BH // N_CORES  # 8
TOK_PER_CORE = SLICES_PER_CORE * S  # 32768
NT = 32                          # tokens per partition per tile
P = 128                          # partitions
TILE_TOK = P * NT                # 4096 tokens = one (b,h) slice
OUT_W = 1 + D + D * D            # 273

R2 = math.sqrt(2.0)
RD = math.sqrt(D)
RRD = math.sqrt(RD)
C_LIN = 1.0 / RRD                # linear-term scale
C_SQ = 1.0 / math.sqrt(R2 * RD)  # prescale: (x_i*C_SQ)*(x_j*C_SQ) = x_i*x_j/(R2*RD)
C_SQ2 = 1.0 / (R2 * RD)          # one-sided: (x_i*C_SQ2)*x_j = x_i*x_j/(R2*RD)

_CACHE = {}


def build_program(reps=1, loop_reps=0, ladder=None, op_bufs=4,
                 load_ring="scalar", prescale_eng="vector",
                 load_order="tile", decouple=False, ot_bf16=True,
                 ot_dt16="float16", ones_eng="gpsimd", hbm_dt="float32",
                 store_ring="sync", layout="ref", sq_mode="plain",
                 sq_loop=False, sq_gp_rows=0, xp_bufs=None, yp_bufs=6,
                 ot_split=False, ydbl_eng="scalar", merge01=False,
                 ones_once=False, yt_eng="scalar"):
    """Build + compile the per-core Bass program. `reps` statically repeats
    the whole pipeline; `loop_reps` wraps it in a hardware For_i loop (both
    used only for HW timing via slope). Non-default values of the remaining
    knobs exist for perf A/B only: `ot_bf16`/`ot_dt16` pick the 16-bit
    output-tile dtype (False = f32 tiles + HWDGE stores), `decouple` makes
    stores read a constant tile instead of the computed one."""
    from contextlib import ExitStack

    import concourse.bacc as bacc
    import concourse.mybir as mybir
    import concourse.tile as tile

    nc = bacc.Bacc("TRN2", target_bir_lowering=False, debug=False)
    hbm_dtype = getattr(mybir.dt, hbm_dt)
    # layout "ref":  row = [1 | x/rrd | sq], width 273 (reference order)
    # layout "pad":  row = [x/rrd | sq | 1 | pad], width 274 — keeps every
    #   fp16 (i, 2J) output pair 4B-aligned so the DVE runs packed 2x mode;
    #   host reorders columns (pure permutation, all values device-computed)
    # layout "tri":  row = [x/rrd | tri | 1 | pad], width 162 — sq is
    #   symmetric (y_i*y_j == y_j*y_i), so store only rows j<=i, each
    #   padded to even length for pair alignment; host mirrors the
    #   duplicate entries (pure gather of device-computed values)
    # layout "trix": exact triangle (136 cols, no even-padding): rows are
    #   reordered so even-length rows come first (all 4B-aligned starts ->
    #   DVE 2x); odd-length rows alternate long(aligned)/short(odd start,
    #   1x -- only lengths 1,3,5,7 pay it); odd-length rows write one
    #   stray element that the next row's op (same engine, in order)
    #   overwrites, so DRAM stays exactly 136 wide. Row i=15 last, clean.
    if layout == "tri":
        tri_len = [(i + 2) // 2 * 2 for i in range(D)]   # 2,2,4,4,...,16,16
        tri_off = [D + sum(tri_len[:i]) for i in range(D)]
        tri_order = list(range(D))
        OW = D + sum(tri_len) + 2                        # 162
    elif layout == "trix":
        tri_order = TRIX_ORDER
        tri_len = [i + 1 for i in range(D)]              # exact lengths
        tri_off = [0] * D
        cur = D
        for i in tri_order:
            tri_off[i] = cur
            cur += tri_len[i]
        OW = cur + 2                                     # 154
    elif layout == "pad":
        OW = OUT_W + 1
    else:
        OW = OUT_W
    x_d = nc.dram_tensor("x", [TOK_PER_CORE, D], mybir.dt.float32,
                         kind="ExternalInput")
    o_d = nc.dram_tensor("out", [TOK_PER_CORE, OW], hbm_dtype,
                         kind="ExternalOutput")

    # flat views: per tile, both input and output regions are contiguous
    x_flat = x_d.ap().rearrange("t d -> (t d)")
    o_flat = o_d.ap().rearrange("t d -> (t d)")

    # Tile-size ladder (tokens per partition per tile): small first tiles so
    # the first out-DMA launches early; 32-token (4.47 MB) tiles in steady
    # state, the probe-measured sweet spot for store throughput.
    if ladder is None:
        ladder = [4, 4, 8, 16] + [NT] * 7
    assert sum(ladder) == TOK_PER_CORE // P
    n_tiles = len(ladder)

    with tile.TileContext(nc) as tc, ExitStack() as ctx:
        xp = ctx.enter_context(tc.tile_pool(
            name="x", bufs=xp_bufs or n_tiles + 1))
        yp = ctx.enter_context(tc.tile_pool(name="y", bufs=yp_bufs))
        op = ctx.enter_context(tc.tile_pool(name="o", bufs=op_bufs))
        cst = None
        if decouple:
            # perf triage: stores read this constant tile instead of the
            # computed one, removing the compute->store dependency
            cp = ctx.enter_context(tc.tile_pool(name="c", bufs=1))
            cst = cp.tile([P, NT * OW], mybir.dt.float32,
                          tag="cst", name="cst")
            nc.gpsimd.memset(cst[:], 1.0)
        if ones_once:
            # the ones/pad columns of every output ring buffer are
            # constant 1.0 at a fixed offset -- initialize each slot once
            # before the loop instead of re-memsetting every tile
            dt16_pre = getattr(mybir.dt, hbm_dt if hbm_dt != "float32"
                               else ot_dt16)
            assert layout in ("pad", "tri", "trix")
            seen = {}
            for nt_ in ladder:
                tag = f"ot{nt_}" if ot_split else "ot"
                nbuf = op_bufs - seen.get(tag, 0)
                seen[tag] = op_bufs
                mx = nt_ if ot_split else max(ladder)
                for _ in range(nbuf):
                    pre = op.tile([P, mx * OW], dt16_pre, tag=tag,
                                  name="ot")
                    pre3 = pre[:].rearrange("p (t f) -> p t f", f=OW)
                    getattr(nc, ones_eng).memset(pre3[:, :, OW - 2:OW], 1.0)
        if loop_reps:
            ctx.enter_context(tc.For_i(0, loop_reps, 1))

        for _ in range(reps):
            xts, pos = [], 0
            if load_order == "front":
                # all input loads queued ahead of the stores on the same
                # ring (xt pool holds one slot per tile)
                for nt_ in ladder:
                    tile_tok = P * nt_
                    xt = xp.tile([P, nt_ * D], mybir.dt.float32, tag="xt",
                                 name="xt")
                    src = x_flat[pos * D:(pos + tile_tok) * D]
                    getattr(nc, load_ring).dma_start(
                        xt[:], src.rearrange("(p f) -> p f", p=P))
                    xts.append(xt)
                    pos += tile_tok
            else:
                xts = [None] * len(ladder)

            # per tile: (load if not front-loaded, then) compute + store
            pos = 0
            if hbm_dt != "float32":
                # 16-bit output straight to HBM: tiles must match hbm dtype
                ot_dt16 = hbm_dt
            dt16 = getattr(mybir.dt, ot_dt16)
            ot_dt = dt16 if ot_bf16 else mybir.dt.float32
            y_dt = dt16 if ot_bf16 else mybir.dt.float32
            for ti, (xt, nt_) in enumerate(zip(xts, ladder)):
                tile_tok = P * nt_
                if xt is None:
                    xt = xp.tile([P, nt_ * D], mybir.dt.float32, tag="xt",
                                 name="xt")
                    src = x_flat[pos * D:(pos + tile_tok) * D]
                    getattr(nc, load_ring).dma_start(
                        xt[:], src.rearrange("(p f) -> p f", p=P))
                yt = yp.tile([P, nt_ * D], y_dt, tag="yt", name="yt")
                ot_tag = f"ot{nt_}" if ot_split else "ot"
                ot = op.tile([P, nt_ * OW], ot_dt, tag=ot_tag, name="ot")

                ot3 = ot[:].rearrange("p (t f) -> p t f", f=OW)
                x3 = xt[:].rearrange("p (t f) -> p t f", f=D)

                if layout in ("pad", "tri", "trix"):
                    lin = ot3[:, :, 0:D]
                    sq_flat = ot3[:, :, D:OW - 2]
                    ones_sl = ot3[:, :, OW - 2:OW]  # ones + pad col
                else:
                    lin = ot3[:, :, 1:1 + D]
                    sq_flat = ot3[:, :, 1 + D:]
                    ones_sl = ot3[:, :, 0:1]

                # ones column (gpsimd by default so DVE/ACT stay free;
                # with ones_once the ring buffers were pre-initialized)
                if not ones_once:
                    getattr(nc, ones_eng).memset(ones_sl, 1.0)

                # linear term on ScalarE: x * C_LIN
                nc.scalar.mul(lin, x3, C_LIN)

                if sq_mode == "pair":
                    # packed-pair outer product: all DVE operands get
                    # innermost step +1 / 4B-aligned so tensor_tensor runs
                    # 2x_1P (2 fp16/cycle) instead of 1x. in0 reads from
                    # ydbl where each y_i appears twice consecutively.
                    ydbl = yp.tile([P, nt_ * 2 * D], y_dt, tag="ydbl",
                                   name="ydbl")
                    yd3 = ydbl[:].rearrange("p (t i pr) -> p t i pr",
                                            i=D, pr=2)
                    xdup = x3.unsqueeze(3).broadcast_to((P, nt_, D, 2))
                    if ydbl_eng == "scalar":
                        nc.scalar.mul(yd3, xdup, C_SQ)
                    else:
                        getattr(nc, ydbl_eng).tensor_scalar_mul(
                            yd3, xdup, C_SQ)
                    if yt_eng == "vector":
                        # yt on DVE (2x_2P tensor_scalar) in parallel with
                        # ACT's ydbl -- shortens the tile-0 critical chain
                        nc.vector.tensor_scalar_mul(yt[:], xt[:], C_SQ)
                    else:
                        nc.scalar.mul(yt[:], xt[:], C_SQ)
                    y4 = yt[:].rearrange("p (t J pr) -> p t J pr",
                                         J=D // 2, pr=2)
                    ndv = D - sq_gp_rows  # i-rows computed by DVE
                    if layout == "trix":
                        # exact triangle: rows in TRIX_ORDER; odd-length
                        # rows write one stray element overwritten by the
                        # next row's op (DVE is in-order, WAW-safe)
                        for i in tri_order:
                            J = (tri_len[i] + 1) // 2
                            out_i = (ot3[:, :,
                                         tri_off[i]:tri_off[i] + 2 * J]
                                     .rearrange("p t (J pr) -> p t J pr",
                                                pr=2))
                            in0 = (yd3[:, :, i:i + 1, :]
                                   .broadcast_to((P, nt_, J, 2)))
                            nc.vector.tensor_mul(out_i, in0,
                                                 y4[:, :, :J, :])
                    elif layout == "tri":
                        # ragged triangle: one op per i-row, j <= i
                        # (padded to even length for pair alignment)
                        i0 = 0
                        if merge01:
                            # rows 0 and 1 (len 2 each) fuse into one op:
                            # out run4 [y0*y0, y0*y1, y1*y0, y1*y1]
                            out01 = (ot3[:, :, tri_off[0]:tri_off[0] + 4]
                                     .rearrange("p t (i pr) -> p t i pr",
                                                pr=2))
                            in1 = (y4[:, :, 0:1, :]
                                   .broadcast_to((P, nt_, 2, 2)))
                            nc.vector.tensor_mul(out01, yd3[:, :, 0:2, :],
                                                 in1)
                            i0 = 2
                        for i in range(i0, D):
                            L = tri_len[i] // 2
                            out_i = (ot3[:, :,
                                         tri_off[i]:tri_off[i] + tri_len[i]]
                                     .rearrange("p t (J pr) -> p t J pr",
                                                pr=2))
                            in0 = (yd3[:, :, i:i + 1, :]
                                   .broadcast_to((P, nt_, L, 2)))
                            nc.vector.tensor_mul(out_i, in0,
                                                 y4[:, :, :L, :])
                    elif sq_loop == "j8":
                        # one op per output column-pair J: in0 = the whole
                        # ydbl tile (fully contiguous, no broadcast), in1 =
                        # the J-th y pair broadcast over i. All APs have
                        # innermost step +1 and 4B-aligned starts -> 2x_1P.
                        sq5 = sq_flat.rearrange(
                            "p t (i J pr) -> p t i J pr", J=D // 2, pr=2)
                        ngj = sq_gp_rows // 2  # J-ops on gpsimd (from top)
                        for Jf in range(D // 2):
                            eng = nc.vector if Jf < D // 2 - ngj \
                                else nc.gpsimd
                            in1 = (y4[:, :, Jf:Jf + 1, :]
                                   .broadcast_to((P, nt_, D, 2)))
                            eng.tensor_mul(sq5[:, :, :, Jf], yd3, in1)
                    elif sq_loop:
                        sq5 = sq_flat.rearrange(
                            "p t (i J pr) -> p t i J pr", J=D // 2, pr=2)
                        for i in range(D):
                            eng = nc.vector if i < ndv else nc.gpsimd
                            in0 = (yd3[:, :, i:i + 1, :]
                                   .broadcast_to((P, nt_, D // 2, 2)))
                            in1 = y4
                            eng.tensor_mul(sq5[:, :, i], in0, in1)
                    else:
                        sq5 = sq_flat.rearrange(
                            "p t (i J pr) -> p t i J pr", J=D // 2, pr=2)
                        in0 = (yd3.unsqueeze(3)
                               .broadcast_to((P, nt_, D, D // 2, 2)))
                        in1 = (y4.unsqueeze(2)
                               .broadcast_to((P, nt_, D, D // 2, 2)))
                        if sq_gp_rows:
                            nc.vector.tensor_mul(
                                sq5[:, :, :ndv], in0[:, :, :ndv],
                                in1[:, :, :ndv])
                            nc.gpsimd.tensor_mul(
                                sq5[:, :, ndv:], in0[:, :, ndv:],
                                in1[:, :, ndv:])
                        else:
                            nc.vector.tensor_mul(sq5, in0, in1)
                else:
                    # prescale y = x * C_SQ (ScalarE by default; DVE then
                    # runs exactly one op per tile, the big outer product)
                    getattr(nc, prescale_eng).mul(yt[:], xt[:], C_SQ) \
                        if prescale_eng == "scalar" else \
                        nc.vector.tensor_scalar_mul(yt[:], xt[:], C_SQ)

                    # outer products: broadcast-AP DVE tensor_tensor
                    y3 = yt[:].rearrange("p (t f) -> p t f", f=D)
                    sq = sq_flat.rearrange("p t (i j) -> p t i j", j=D)
                    ndv = D - sq_gp_rows
                    in0 = y3.unsqueeze(3).broadcast_to((P, nt_, D, D))
                    in1 = y3.unsqueeze(2).broadcast_to((P, nt_, D, D))
                    if sq_gp_rows:
                        nc.vector.tensor_mul(
                            sq[:, :, :ndv], in0[:, :, :ndv],
                            in1[:, :, :ndv])
                        nc.gpsimd.tensor_mul(
                            sq[:, :, ndv:], in0[:, :, ndv:],
                            in1[:, :, ndv:])
                    else:
                        nc.vector.tensor_mul(sq, in0, in1)

                # store: contiguous (up to 4.47 MB) on the SP ring. With a
                # bf16 output tile the store goes via SWDGE (gpsimd), which
                # upcasts bf16->f32 inline during the DMA; HBM still
                # receives the full f32 output.
                dst = o_flat[pos * OW:(pos + tile_tok) * OW]
                src_t = cst[:, :nt_ * OW] if decouple else ot[:]
                if ot_bf16 and hbm_dt == "float32":
                    # 16-bit tile, f32 HBM: SWDGE casts inline during DMA
                    nc.gpsimd.dma_start(
                        dst.rearrange("(p f) -> p f", p=P), src_t)
                else:
                    # dtypes match: plain HWDGE store
                    ring = store_ring
                    if ring == "alt":  # alternate SP / ACT HWDGE rings
                        ring = "sync" if ti % 2 == 0 else "scalar"
                    getattr(nc, ring).dma_start(
                        dst.rearrange("(p f) -> p f", p=P), src_t)
                pos += tile_tok

    nc.compile()
    return nc


def _make_runner(nc):
    """One-time: build a cached jitted shard_map executor for `nc`."""
    import jax
    from jax.experimental.shard_map import shard_map
    from jax.sharding import Mesh, NamedSharding, PartitionSpec

    import concourse.mybir as mybir
    from concourse.bass2jax import (
        _bass_exec_p,
        install_neuronx_cc_hook,
        partition_id_tensor,
    )

    install_neuronx_cc_hook()

    in_names, out_names, out_avals = [], [], []
    pname = nc.partition_id_tensor.name if nc.partition_id_tensor else None
    for alloc in nc.m.functions[0].allocations:
        if not isinstance(alloc, mybir.MemoryLocationSet):
            continue
        name = alloc.memorylocations[0].name
        if alloc.kind == "ExternalInput":
            if name != pname:
                in_names.append(name)
        elif alloc.kind == "ExternalOutput":
            out_names.append(name)
            out_avals.append(jax.core.ShapedArray(
                tuple(alloc.tensor_shape), mybir.dt.np(alloc.dtype)))
    assert in_names == ["x"] and out_names == ["out"], (in_names, out_names)

    all_in = tuple(in_names) + tuple(out_names)
    if pname is not None:
        all_in = all_in + (pname,)
    bind_kwargs = dict(
        out_avals=tuple(out_avals),
        in_names=all_in,
        out_names=tuple(out_names),
        lowering_input_output_aliases=(),
        sim_require_finite=True,
        sim_require_nnan=True,
        nc=nc,
    )

    def _body(x, obuf):
        operands = [x, obuf]
        if pname is not None:
            operands.append(partition_id_tensor())
        (o,) = _bass_exec_p.bind(*operands, **bind_kwargs)
        return (o,)

    mesh = Mesh(np.asarray(jax.devices()[:N_CORES]), ("core",))
    fn = jax.jit(
        shard_map(_body, mesh=mesh,
                  in_specs=(PartitionSpec("core"), PartitionSpec("core")),
                  out_specs=(PartitionSpec("core"),),
                  check_rep=False),
        donate_argnums=(1,),
    )
    sharding = NamedSharding(mesh, PartitionSpec("core"))
    oshape = (N_CORES * out_avals[0].shape[0],) + tuple(out_avals[0].shape[1:])
    odtype = out_avals[0].dtype

    make_zeros = jax.jit(lambda: jax.numpy.zeros(oshape, odtype),
                         out_shardings=sharding)

    def run(x_concat: np.ndarray) -> np.ndarray:
        x_dev = jax.device_put(x_concat, sharding)
        (o,) = fn(x_dev, make_zeros())
        return np.asarray(o)

    return run


def _run_spmd_fallback(nc, x2: np.ndarray) -> np.ndarray:
    """Canonical path: bass_utils.run_bass_kernel_spmd (works both under
    axon/PJRT and with native /dev/neuron* NRT)."""
    from concourse.bass_utils import run_bass_kernel_spmd

    in_maps = [
        {"x": x2[c * TOK_PER_CORE:(c + 1) * TOK_PER_CORE]}
        for c in range(N_CORES)
    ]
    res = run_bass_kernel_spmd(nc, in_maps, core_ids=list(range(N_CORES)))
    return np.concatenate([r["out"] for r in res.results], axis=0)


# triangle layout tables (mirrors build_program's layout="tri")
TRI_LEN = [(i + 2) // 2 * 2 for i in range(D)]
TRI_OFF = [D + sum(TRI_LEN[:i]) for i in range(D)]

# exact-triangle ("trix") row order: even-length rows first, then
# odd-length rows long/short interleaved, i=15 last (see build_program)
TRIX_ORDER = [1, 3, 5, 7, 9, 11, 13] + [14, 0, 12, 2, 10, 4, 8, 6] + [15]


TRIX_OFF = [0] * D
_cur = D
for _i in TRIX_ORDER:
    TRIX_OFF[_i] = _cur
    _cur += _i + 1


def _sym_index(off, length):
    """Device column holding ref sq element (i, j) in a triangle layout."""
    idx = np.empty(D * D, np.int64)
    for f in range(D * D):
        i, j = f // D, f % D
        idx[f] = off[i] + j if j < length[i] else off[j] + i
    return idx


_SYM = _sym_index(TRI_OFF, TRI_LEN)
_SYM_X = _sym_index(TRIX_OFF, [i + 1 for i in range(D)])

# kernel() build configuration (selected by measurement; see module docstring)
BEST_CFG = {
    "hbm_dt": "float16",
    "layout": "tri",
    "sq_mode": "pair",
    "sq_loop": "i16",     # ignored for layout="tri" (tri has its own loop)
    "op_bufs": 3,
    "ot_split": True,
    "ladder": [32, 48, 48, 48, 48, 32],
}


def kernel(x: np.ndarray) -> np.ndarray:
    x = np.ascontiguousarray(np.asarray(x, dtype=np.float32))
    assert x.shape == (B, H, S, D), x.shape

    if "nc" not in _CACHE:
        _CACHE["nc"] = build_program(**BEST_CFG)
        try:
            from concourse._compat import axon_active
            _CACHE["run"] = (_make_runner(_CACHE["nc"])
                             if axon_active() else None)
        except Exception:
            _CACHE["run"] = None

    # core c gets (b,h) slices [8c, 8c+8) -> concat over cores is just
    # the natural [BH*S, D] layout
    x2 = x.reshape(BH * S, D)
    out = None
    if _CACHE.get("run") is not None:
        try:
            out = _CACHE["run"](x2)      # cached fast path (axon/PJRT)
        except Exception:
            _CACHE["run"] = None
    if out is None:
        out = _run_spmd_fallback(_CACHE["nc"], x2)
    layout = BEST_CFG.get("layout", "ref")
    if layout in ("pad", "tri", "trix"):
        # device row = [x/rrd (16) | sq | 1 | pad]; reassemble the
        # reference column order on host — a pure gather/permutation +
        # f32 upcast of device-computed values ("tri" additionally
        # mirrors each symmetric pair from its single device copy)
        full = np.empty((BH * S, OUT_W), np.float32)
        full[:, 0] = out[:, -2]
        full[:, 1:1 + D] = out[:, 0:D]
        if layout == "tri":
            full[:, 1 + D:] = out[:, _SYM]
        elif layout == "trix":
            full[:, 1 + D:] = out[:, _SYM_X]
        else:
            full[:, 1 + D:] = out[:, D:D + D * D]
        out = full
    elif out.dtype != np.float32:
        out = np.asarray(out, dtype=np.float32)
    return out.reshape(B, H, S, OUT_W)

